# revision 5
# baseline (speedup 1.0000x reference)
"""KNN-attention layer on 8 Trainium2 NeuronCores (Bass/Tile).

Sharding: core c owns query rows [256c, 256c+256) (batch c//4) and store
shard [4096c, 4096(c+1)).

Per-core program (static, identical on all cores):
  P0  knn-query projection (bf16 hi/lo split-3 matmul for fp32-exact sims),
      normalize, transpose, AllGather qn across cores.
  P1  per store half (2048): normalize+split+transpose keys; sims = qn @ kn.T
      for all 2048 rows via split-3; spill sims to DRAM; local top-32 per half
      (max8 + match_replace).
  P2  merge halves -> local top-32; AllGather candidates; global threshold
      t (32nd) and max m per row.
  P3  per half: reload sims, W = (s >= t) * exp((s-m)/T) in bf16 (+Z accum),
      transpose W, weighted matmul with store_vals -> unnormalized knn partial
      [1024, 2048]; pack + ReduceScatter (sum over cores, scatter by row block).
  P4  causal self-attention for own row block (bf16), static chunking:
      3 absolute 256-wide key pre-chunks gated by a per-core validity column
      + 1 diagonal chunk from the own-block x slice.
  P5  normalize knn by Z, project (+bias), gated combine, transpose to
      row-major, output [256, 1024] per core.

Selection precision: sims are computed as q_hi@k_hi + q_hi@k_lo + q_lo@k_hi
(bf16 splits, fp32 PSUM accumulation) which matches fp32 sims to ~1e-7 —
required because the global top-32 set must match the fp32 reference's.
"""
import sys

sys.path.insert(0, "/opt/trn_rl_repo")

import numpy as np

# ---------------- problem constants ----------------
B, S, D = 2, 1024, 1024
H, HD = 16, 64
N = 32768
KNN_K = 32
TEMP = 0.1
NCORES = 8
ROWS = B * S                # 2048
RPC = ROWS // NCORES        # 256 rows per core
SS = N // NCORES            # 4096 stores per core
HNS = SS // 2               # 2048 per half
DT = D // 128               # 8 feature tiles
RT = ROWS // 128            # 16 global row tiles
MT = RPC // 128             # 2 own row tiles
HIT = HNS // 128            # 16 store tiles per half
NEG = -1e30
RG = [list(range(NCORES))]

_BUILT = {}


def _build(trace=False):
    if "nc" in _BUILT:
        return _BUILT["nc"]
    from contextlib import ExitStack
    import concourse.bass as bass
    import concourse.tile as tile
    from concourse import bacc, mybir
    from concourse.masks import make_identity

    f32 = mybir.dt.float32
    bf16 = mybir.dt.bfloat16
    AOT = mybir.AluOpType
    AF = mybir.ActivationFunctionType

    nc = bacc.Bacc("TRN2", target_bir_lowering=False, debug=False,
                   num_devices=NCORES)

    # ---- I/O ----
    XT = nc.dram_tensor("xt", [D, S], f32, kind="ExternalInput").ap()
    XTQ = nc.dram_tensor("xtq", [D, RPC], f32, kind="ExternalInput").ap()
    KEYS = nc.dram_tensor("keys", [SS, D], f32, kind="ExternalInput").ap()
    VALS = nc.dram_tensor("vals", [SS, D], f32, kind="ExternalInput").ap()
    WQT = nc.dram_tensor("wqt", [D, D], f32, kind="ExternalInput").ap()
    WKT = nc.dram_tensor("wkt", [D, D], f32, kind="ExternalInput").ap()
    WVT = nc.dram_tensor("wvt", [D, D], f32, kind="ExternalInput").ap()
    WOT = nc.dram_tensor("wot", [D, D], f32, kind="ExternalInput").ap()
    WKKT = nc.dram_tensor("wkkt", [D, D], f32, kind="ExternalInput").ap()
    WPT = nc.dram_tensor("wpt", [D, D], f32, kind="ExternalInput").ap()
    WGT = nc.dram_tensor("wgt", [2 * D, 1], f32, kind="ExternalInput").ap()
    BG = nc.dram_tensor("bg", [1, 1], f32, kind="ExternalInput").ap()
    BPROJ = nc.dram_tensor("bproj", [D, 1], f32, kind="ExternalInput").ap()
    VALIDN = nc.dram_tensor("validn", [128, 3], f32, kind="ExternalInput").ap()
    OUT = nc.dram_tensor("out", [RPC, D], f32, kind="ExternalOutput").ap()

    # ---- DRAM scratch ----
    simsbuf = nc.dram_tensor("simsbuf", [RT, 2, 128, HNS], f32,
                             kind="Internal").ap()
    qn_b = nc.dram_tensor("qn_b", [2, D, RPC], bf16, kind="Internal").ap()
    qn_g = nc.dram_tensor("qn_g", [NCORES, 2, D, RPC], bf16, kind="Internal",
                          addr_space="Shared").ap()
    cand_b = nc.dram_tensor("cand_b", [RT, 128, 32], f32, kind="Internal").ap()
    cand_g = nc.dram_tensor("cand_g", [NCORES, RT, 128, 32], f32,
                            kind="Internal", addr_space="Shared").ap()
    rs_in = nc.dram_tensor("rs_in", [NCORES, 1152, RPC], f32,
                           kind="Internal").ap()
    rs_out = nc.dram_tensor("rs_out", [1152, RPC], f32, kind="Internal").ap()

    def rsqrt_newton(pool, n2, tagp):
        r0 = pool.tile([128, 1], f32, tag=tagp + "r0")
        nc.vector.reciprocal(r0[:], n2)
        y = pool.tile([128, 1], f32, tag=tagp + "y")
        nc.scalar.activation(y[:], r0[:], AF.Sqrt)
        for it in ("a", "b"):
            yy = pool.tile([128, 1], f32, tag=tagp + it + "1")
            nc.vector.tensor_tensor(yy[:], y[:], y[:], op=AOT.mult)
            nc.vector.tensor_tensor(yy[:], n2, yy[:], op=AOT.mult)
            nc.vector.tensor_scalar(yy[:], yy[:], -0.5, scalar2=1.5,
                                    op0=AOT.mult, op1=AOT.add)
            y2 = pool.tile([128, 1], f32, tag=tagp + it + "2")
            nc.vector.tensor_tensor(y2[:], y[:], yy[:], op=AOT.mult)
            y = y2
        return y

    def split_hi_lo(pool, src, hi, lo, tagp):
        nc.vector.tensor_copy(hi, src)
        h32 = pool.tile(list(src.shape), f32, tag=tagp + "h32")
        nc.vector.tensor_copy(h32[:], hi)
        nc.vector.tensor_tensor(h32[:], src, h32[:], op=AOT.subtract)
        nc.vector.tensor_copy(lo, h32[:])

    with tile.TileContext(nc) as tc, ExitStack() as ctx:
        res = ctx.enter_context(tc.tile_pool(name="res", bufs=1))
        ident = res.tile([128, 128], bf16)
        make_identity(nc, ident)
        identf = res.tile([128, 128], f32)
        make_identity(nc, identf)
        maxes_all = res.tile([128, RT, 64], f32)
        t_m = res.tile([128, RT, 2], f32)
        attn_oT = res.tile([128, DT, RPC], f32)
        attn_oTb = res.tile([128, DT, RPC], bf16)

        # ================= P0: qn + AllGather =================
        with tc.tile_pool(name="p0", bufs=2) as p0, \
             tc.tile_pool(name="ps0", bufs=2, space="PSUM") as ps0, \
             tc.tile_pool(name="ps0t", bufs=2, space="PSUM") as ps0t:
            xq_hi = p0.tile([128, DT, RPC], bf16, tag="xqh")
            xq_lo = p0.tile([128, DT, RPC], bf16, tag="xql")
            wk_hi = p0.tile([128, DT, D], bf16, tag="wkh")
            wk_lo = p0.tile([128, DT, D], bf16, tag="wkl")
            for d in range(DT):
                t = p0.tile([128, RPC], f32, tag="ld")
                nc.sync.dma_start(t[:], XTQ[d * 128:(d + 1) * 128, :])
                split_hi_lo(p0, t[:], xq_hi[:, d, :], xq_lo[:, d, :], "sx")
                tw = p0.tile([128, D], f32, tag="ldw")
                nc.sync.dma_start(tw[:], WKKT[d * 128:(d + 1) * 128, :])
                split_hi_lo(p0, tw[:], wk_hi[:, d, :], wk_lo[:, d, :], "sw")
            for m in range(MT):
                qrow = p0.tile([128, D], f32, tag="qrow")
                for chn in range(2):
                    ps = ps0.tile([128, 512], f32, tag="mm")
                    cs = slice(chn * 512, (chn + 1) * 512)
                    ms = slice(m * 128, (m + 1) * 128)
                    for d in range(DT):
                        nc.tensor.matmul(ps[:], xq_hi[:, d, ms], wk_hi[:, d, cs],
                                         start=(d == 0), stop=False)
                        nc.tensor.matmul(ps[:], xq_hi[:, d, ms], wk_lo[:, d, cs],
                                         start=False, stop=False)
                        nc.tensor.matmul(ps[:], xq_lo[:, d, ms], wk_hi[:, d, cs],
                                         start=False, stop=(d == DT - 1))
                    nc.vector.tensor_copy(qrow[:, cs], ps[:])
                n2 = p0.tile([128, 1], f32, tag="n2")
                sq = p0.tile([128, D], f32, tag="sq")
                nc.scalar.activation(sq[:], qrow[:], AF.Square, accum_out=n2[:])
                inv = rsqrt_newton(p0, n2[:], "nq")
                nc.vector.tensor_scalar_mul(qrow[:], qrow[:], inv[:])
                q_hi = p0.tile([128, D], bf16, tag="qhi")
                q_lo = p0.tile([128, D], bf16, tag="qlo")
                split_hi_lo(p0, qrow[:], q_hi[:], q_lo[:], "sq2")
                for d in range(DT):
                    for hl, src in enumerate((q_hi, q_lo)):
                        tp = ps0t.tile([128, 128], bf16, tag="tp")
                        nc.tensor.transpose(tp[:], src[:, d * 128:(d + 1) * 128],
                                            ident[:])
                        sb = p0.tile([128, 128], bf16, tag="tpo")
                        nc.vector.tensor_copy(sb[:], tp[:])
                        nc.sync.dma_start(
                            qn_b[hl, d * 128:(d + 1) * 128,
                                 m * 128:(m + 1) * 128], sb[:])
            nc.gpsimd.collective_compute(
                "AllGather", AOT.bypass, replica_groups=RG,
                ins=[qn_b.opt()], outs=[qn_g.opt()])

        # ================= P1: keys + sims + local topk =================
        for h in range(2):
            with tc.tile_pool(name=f"p1_{h}", bufs=1) as p1, \
                 tc.tile_pool(name=f"p1b_{h}", bufs=2) as p1b, \
                 tc.tile_pool(name=f"ps1_{h}", bufs=4, space="PSUM") as ps1, \
                 tc.tile_pool(name=f"ps1t_{h}", bufs=2, space="PSUM") as ps1t:
                knT_hi = p1.tile([128, DT, HNS], bf16, tag="knh")
                knT_lo = p1.tile([128, DT, HNS], bf16, tag="knl")
                for i in range(HIT):
                    kt = p1b.tile([128, D], f32, tag="kld")
                    nc.sync.dma_start(
                        kt[:], KEYS[h * HNS + i * 128:h * HNS + (i + 1) * 128, :])
                    n2 = p1b.tile([128, 1], f32, tag="kn2")
                    sq = p1b.tile([128, D], f32, tag="ksq")
                    nc.scalar.activation(sq[:], kt[:], AF.Square, accum_out=n2[:])
                    inv = rsqrt_newton(p1b, n2[:], "nk")
                    nc.vector.tensor_scalar_mul(kt[:], kt[:], inv[:])
                    k_hi = p1b.tile([128, D], bf16, tag="khi")
                    k_lo = p1b.tile([128, D], bf16, tag="klo")
                    split_hi_lo(p1b, kt[:], k_hi[:], k_lo[:], "sk")
                    for d in range(DT):
                        for src, dst in ((k_hi, knT_hi), (k_lo, knT_lo)):
                            tp = ps1t.tile([128, 128], bf16, tag="tp")
                            nc.tensor.transpose(
                                tp[:], src[:, d * 128:(d + 1) * 128], ident[:])
                            nc.vector.tensor_copy(
                                dst[:, d, i * 128:(i + 1) * 128], tp[:])
                for rt in range(RT):
                    cb, ml = rt // 2, rt % 2
                    qh = p1b.tile([128, DT, 128], bf16, tag="qsh")
                    ql = p1b.tile([128, DT, 128], bf16, tag="qsl")
                    for hl, dst in ((0, qh), (1, ql)):
                        nc.sync.dma_start(
                            dst[:],
                            qn_g[cb, hl, :, ml * 128:(ml + 1) * 128].rearrange(
                                "(dt p) c -> p dt c", p=128))
                    sims = p1b.tile([128, HNS], f32, tag="sims")
                    for chn in range(HNS // 512):
                        ps = ps1.tile([128, 512], f32, tag="mm")
                        cs = slice(chn * 512, (chn + 1) * 512)
                        for d in range(DT):
                            nc.tensor.matmul(ps[:], qh[:, d, :], knT_hi[:, d, cs],
                                             start=(d == 0), stop=False)
                            nc.tensor.matmul(ps[:], qh[:, d, :], knT_lo[:, d, cs],
                                             start=False, stop=False)
                            nc.tensor.matmul(ps[:], ql[:, d, :], knT_hi[:, d, cs],
                                             start=False, stop=(d == DT - 1))
                        nc.scalar.activation(sims[:, cs], ps[:], AF.Copy)
                    nc.sync.dma_start(simsbuf[rt, h], sims[:])
                    scr = p1b.tile([128, HNS], f32, tag="scr")
                    nc.vector.tensor_copy(scr[:], sims[:])
                    for r in range(4):
                        mx = maxes_all[:, rt, h * 32 + r * 8:h * 32 + (r + 1) * 8]
                        nc.vector.max(mx, scr[:])
                        nc.vector.match_replace(scr[:], mx, scr[:], NEG)

        # ================= P2: merge + AllGather candidates ===============
        with tc.tile_pool(name="p2", bufs=2) as p2, \
             tc.tile_pool(name="ps2", bufs=2, space="PSUM") as ps2:
            for rt in range(RT):
                scr = p2.tile([128, 64], f32, tag="mscr")
                nc.vector.tensor_copy(scr[:], maxes_all[:, rt, :])
                loc = p2.tile([128, 32], f32, tag="loc")
                for r in range(4):
                    nc.vector.max(loc[:, r * 8:(r + 1) * 8], scr[:])
                    nc.vector.match_replace(scr[:], loc[:, r * 8:(r + 1) * 8],
                                            scr[:], NEG)
                nc.sync.dma_start(cand_b[rt], loc[:])
            nc.gpsimd.collective_compute(
                "AllGather", AOT.bypass, replica_groups=RG,
                ins=[cand_b.opt()], outs=[cand_g.opt()])
            for rt in range(RT):
                gall = p2.tile([128, NCORES * 32], f32, tag="gall")
                for cb in range(NCORES):
                    nc.sync.dma_start(gall[:, cb * 32:(cb + 1) * 32],
                                      cand_g[cb, rt])
                gm = p2.tile([128, 4, 8], f32, tag="gm")
                for r in range(4):
                    nc.vector.max(gm[:, r, :], gall[:])
                    nc.vector.match_replace(gall[:], gm[:, r, :], gall[:], NEG)
                nc.vector.tensor_copy(t_m[:, rt, 0:1], gm[:, 3, 7:8])
                nc.vector.tensor_copy(t_m[:, rt, 1:2], gm[:, 0, 0:1])

        # ================= P3: W, weighted matmul, ReduceScatter ==========
        with tc.tile_pool(name="p3", bufs=1) as p3, \
             tc.tile_pool(name="p3b", bufs=2) as p3b, \
             tc.tile_pool(name="ps3", bufs=4, space="PSUM") as ps3, \
             tc.tile_pool(name="ps3t", bufs=2, space="PSUM") as ps3t:
            unorm = p3.tile([128, DT, ROWS], f32, tag="unorm")
            z_cols = p3.tile([128, RT], f32, tag="zc")
            for h in range(2):
                vals_bf = p3.tile([128, HIT, D], bf16, tag="vbf")
                for i in range(HIT):
                    vt = p3b.tile([128, D], f32, tag="vld")
                    nc.sync.dma_start(
                        vt[:], VALS[h * HNS + i * 128:h * HNS + (i + 1) * 128, :])
                    nc.vector.tensor_copy(vals_bf[:, i, :], vt[:])
                for g in range(4):
                    w_T = p3b.tile([128, HIT, 512], bf16, tag="wT")
                    for rl in range(4):
                        rt = g * 4 + rl
                        sims = p3b.tile([128, HNS], f32, tag="srl")
                        nc.sync.dma_start(sims[:], simsbuf[rt, h])
                        mbias = p3b.tile([128, 1], f32, tag="mb")
                        nc.vector.tensor_scalar_mul(mbias[:], t_m[:, rt, 1:2],
                                                    -1.0 / TEMP)
                        expw = p3b.tile([128, HNS], f32, tag="expw")
                        nc.scalar.activation(expw[:], sims[:], AF.Exp,
                                             bias=mbias[:], scale=1.0 / TEMP)
                        wmask = p3b.tile([128, HNS], bf16, tag="wm")
                        z = p3b.tile([128, 1], f32, tag="z")
                        nc.vector.scalar_tensor_tensor(
                            wmask[:], sims[:], t_m[:, rt, 0:1], expw[:],
                            op0=AOT.is_ge, op1=AOT.mult, accum_out=z[:])
                        if h == 0:
                            nc.vector.tensor_copy(z_cols[:, rt:rt + 1], z[:])
                        else:
                            nc.vector.tensor_tensor(z_cols[:, rt:rt + 1],
                                                    z_cols[:, rt:rt + 1], z[:],
                                                    op=AOT.add)
                        for i in range(HIT):
                            tp = ps3t.tile([128, 128], bf16, tag="tp")
                            nc.tensor.transpose(
                                tp[:], wmask[:, i * 128:(i + 1) * 128], ident[:])
                            nc.vector.tensor_copy(
                                w_T[:, i, rl * 128:(rl + 1) * 128], tp[:])
                    for d in range(DT):
                        ps = ps3.tile([128, 512], f32, tag="mm")
                        for i in range(HIT):
                            nc.tensor.matmul(
                                ps[:], vals_bf[:, i, d * 128:(d + 1) * 128],
                                w_T[:, i, :], start=(i == 0), stop=(i == HIT - 1))
                        gs = slice(g * 512, (g + 1) * 512)
                        if h == 0:
                            nc.scalar.activation(unorm[:, d, gs], ps[:], AF.Copy)
                        else:
                            nc.vector.tensor_tensor(unorm[:, d, gs],
                                                    unorm[:, d, gs], ps[:],
                                                    op=AOT.add)
            # pack rs_in: [cb][dt*128+p][lc] = unorm[p, dt, cb*256+lc]
            for cb in range(NCORES):
                for d in range(DT):
                    nc.sync.dma_start(
                        rs_in[cb, d * 128:(d + 1) * 128, :],
                        unorm[:, d, cb * RPC:(cb + 1) * RPC])
            # z row: transpose z_cols [128, 16] -> [16, 128]
            zt = ps3t.tile([RT, 128], f32, tag="zt")
            nc.tensor.transpose(zt[:], z_cols[:], identf[:])
            zrow = p3.tile([RT, 128], f32, tag="zrow")
            nc.vector.tensor_copy(zrow[:], zt[:])
            zero = p3.tile([128, RPC], f32, tag="zero")
            nc.vector.memset(zero[:], 0.0)
            for cb in range(NCORES):
                nc.sync.dma_start(
                    rs_in[cb, 1024:1025, :].rearrange("o (a b) -> (o a) b", a=2),
                    zrow[cb * 2:cb * 2 + 2, :])
                nc.sync.dma_start(rs_in[cb, 1025:1152, :], zero[0:127, :])
            nc.gpsimd.collective_compute(
                "ReduceScatter", AOT.add, replica_groups=RG,
                ins=[rs_in.opt()], outs=[rs_out.opt()])

        # ================= P4: causal attention (own block) ===============
        with tc.tile_pool(name="p4", bufs=1) as p4, \
             tc.tile_pool(name="p4b", bufs=2) as p4b, \
             tc.tile_pool(name="ps4", bufs=2, space="PSUM") as ps4, \
             tc.tile_pool(name="ps4s", bufs=2, space="PSUM") as ps4s, \
             tc.tile_pool(name="ps4p", bufs=2, space="PSUM") as ps4p, \
             tc.tile_pool(name="ps4t", bufs=2, space="PSUM") as ps4t:
            xb = p4.tile([128, DT, S], bf16, tag="xb")
            xqb = p4.tile([128, DT, RPC], bf16, tag="xqb")
            wq = p4.tile([128, DT, D], bf16, tag="wq")
            wk = p4.tile([128, DT, D], bf16, tag="wk")
            wv = p4.tile([128, DT, D], bf16, tag="wv")
            wo = p4.tile([128, DT, D], bf16, tag="wo")
            vneg = p4.tile([128, 3], f32, tag="vneg")
            nc.sync.dma_start(vneg[:], VALIDN[:])
            for d in range(DT):
                t = p4b.tile([128, S], f32, tag="xld")
                nc.sync.dma_start(t[:], XT[d * 128:(d + 1) * 128, :])
                nc.vector.tensor_copy(xb[:, d, :], t[:])
                tq = p4b.tile([128, RPC], f32, tag="xqld")
                nc.sync.dma_start(tq[:], XTQ[d * 128:(d + 1) * 128, :])
                nc.vector.tensor_copy(xqb[:, d, :], tq[:])
                for src, dst in ((WQT, wq), (WKT, wk), (WVT, wv), (WOT, wo)):
                    tw = p4b.tile([128, D], f32, tag="wld")
                    nc.sync.dma_start(tw[:], src[d * 128:(d + 1) * 128, :])
                    nc.vector.tensor_copy(dst[:, d, :], tw[:])
            # qT from xqb; kT pre [0,768) from xb; kT diag from xqb
            qT = p4.tile([128, DT, RPC], bf16, tag="qT")
            kTp = p4.tile([128, DT, 768], bf16, tag="kTp")
            kTd = p4.tile([128, DT, RPC], bf16, tag="kTd")
            for d in range(DT):
                ps = ps4.tile([128, 512], f32, tag="pj")
                nc_d = slice(d * 128, (d + 1) * 128)
                for e in range(DT):
                    nc.tensor.matmul(ps[:, :RPC], wq[:, e, nc_d], xqb[:, e, :],
                                     start=(e == 0), stop=(e == DT - 1))
                nc.vector.tensor_copy(qT[:, d, :], ps[:, :RPC])
                for e in range(DT):
                    nc.tensor.matmul(ps[:, :RPC], wk[:, e, nc_d], xqb[:, e, :],
                                     start=(e == 0), stop=(e == DT - 1))
                nc.vector.tensor_copy(kTd[:, d, :], ps[:, :RPC])
                for chn in range(2):
                    cs = slice(chn * 512, min((chn + 1) * 512, 768))
                    w_ = cs.stop - cs.start
                    for e in range(DT):
                        nc.tensor.matmul(ps[:, :w_], wk[:, e, nc_d],
                                         xb[:, e, cs], start=(e == 0),
                                         stop=(e == DT - 1))
                    nc.vector.tensor_copy(kTp[:, d, cs], ps[:, :w_])
            # v rows: pre [0,768) (6 tiles) from xb cols, diag (2 tiles) from xqb
            vR = p4.tile([128, 6, D], bf16, tag="vR")
            vRd = p4.tile([128, MT, D], bf16, tag="vRd")
            for k in range(6):
                for chn in range(2):
                    ps = ps4.tile([128, 512], f32, tag="pj")
                    cs = slice(chn * 512, (chn + 1) * 512)
                    for e in range(DT):
                        nc.tensor.matmul(ps[:], xb[:, e, k * 128:(k + 1) * 128],
                                         wv[:, e, cs], start=(e == 0),
                                         stop=(e == DT - 1))
                    nc.vector.tensor_copy(vR[:, k, cs], ps[:])
            for m in range(MT):
                for chn in range(2):
                    ps = ps4.tile([128, 512], f32, tag="pj")
                    cs = slice(chn * 512, (chn + 1) * 512)
                    for e in range(DT):
                        nc.tensor.matmul(ps[:], xqb[:, e, m * 128:(m + 1) * 128],
                                         wv[:, e, cs], start=(e == 0),
                                         stop=(e == DT - 1))
                    nc.vector.tensor_copy(vRd[:, m, cs], ps[:])
            acT = p4.tile([128, DT, RPC], bf16, tag="acT")
            for hh in range(H):
                d, half = hh // 2, hh % 2
                hp = slice(64 * half, 64 * half + 64)
                for m in range(MT):
                    sc = p4b.tile([128, 1024], f32, tag="sc")
                    for chn in range(3):      # pre-chunks, absolute [chn*256,..)
                        ps = ps4s.tile([128, 256], f32, tag="sps")
                        nc.tensor.matmul(ps[:], qT[hp, d, m * 128:(m + 1) * 128],
                                         kTp[hp, d, chn * 256:(chn + 1) * 256])
                        nc.vector.tensor_scalar(
                            sc[:, chn * 256:(chn + 1) * 256], ps[:],
                            vneg[:, chn:chn + 1], scalar2=None, op0=AOT.add)
                    ps = ps4s.tile([128, 256], f32, tag="sps")
                    nc.tensor.matmul(ps[:], qT[hp, d, m * 128:(m + 1) * 128],
                                     kTd[hp, d, :])
                    nc.scalar.activation(sc[:, 768:1024], ps[:], AF.Copy)
                    # causal on diag: q = m*128 + p (block-local), k = j
                    nc.gpsimd.affine_select(
                        sc[:, 768:1024], sc[:, 768:1024],
                        pattern=[[-1, 256]], compare_op=AOT.is_ge, fill=NEG,
                        base=m * 128, channel_multiplier=1)
                    rm = p4b.tile([128, 1], f32, tag="rm")
                    nc.vector.tensor_reduce(rm[:], sc[:],
                                            axis=mybir.AxisListType.XYZW,
                                            op=AOT.max)
                    nc.vector.tensor_scalar_mul(rm[:], rm[:], -0.125)
                    ex = p4b.tile([128, 1024], f32, tag="ex")
                    zr = p4b.tile([128, 1], f32, tag="zr")
                    nc.scalar.activation(ex[:], sc[:], AF.Exp, bias=rm[:],
                                         scale=0.125, accum_out=zr[:])
                    iz = p4b.tile([128, 1], f32, tag="iz")
                    nc.vector.reciprocal(iz[:], zr[:])
                    ab = p4b.tile([128, 1024], bf16, tag="ab")
                    nc.scalar.activation(ab[:], ex[:], AF.Copy, scale=iz[:])
                    pv = ps4p.tile([128, 128], f32, tag="pv")
                    for k in range(8):
                        tp = ps4t.tile([128, 128], bf16, tag="tp")
                        nc.tensor.transpose(tp[:], ab[:, k * 128:(k + 1) * 128],
                                            ident[:])
                        at = p4b.tile([128, 128], bf16, tag="at")
                        nc.vector.tensor_copy(at[:], tp[:])
                        vsrc = (vR[:, k, hh * 64:(hh + 1) * 64] if k < 6 else
                                vRd[:, k - 6, hh * 64:(hh + 1) * 64])
                        nc.tensor.matmul(pv[hp, :], vsrc, at[:],
                                         start=(k == 0), stop=(k == 7))
                    nc.vector.tensor_copy(acT[hp, d, m * 128:(m + 1) * 128],
                                          pv[hp, :])
            for d in range(DT):
                ps = ps4.tile([128, 512], f32, tag="pj")
                for e in range(DT):
                    nc.tensor.matmul(ps[:, :RPC], wo[:, e, d * 128:(d + 1) * 128],
                                     acT[:, e, :], start=(e == 0),
                                     stop=(e == DT - 1))
                nc.vector.tensor_copy(attn_oT[:, d, :], ps[:, :RPC])
                nc.vector.tensor_copy(attn_oTb[:, d, :], ps[:, :RPC])

        # ================= P5: finale =================
        with tc.tile_pool(name="p5", bufs=1) as p5, \
             tc.tile_pool(name="p5b", bufs=2) as p5b, \
             tc.tile_pool(name="ps5", bufs=2, space="PSUM") as ps5, \
             tc.tile_pool(name="ps5t", bufs=2, space="PSUM") as ps5t:
            knn_s = p5.tile([128, DT, RPC], f32, tag="kns")
            for d in range(DT):
                nc.sync.dma_start(knn_s[:, d, :],
                                  rs_out[d * 128:(d + 1) * 128, :])
            zg = p5.tile([1, RPC], f32, tag="zg")
            nc.sync.dma_start(zg[:], rs_out[1024:1025, :])
            iz = p5.tile([1, RPC], f32, tag="izg")
            nc.vector.reciprocal(iz[:], zg[:])
            ones_r = p5.tile([1, 128], f32, tag="ones")
            nc.vector.memset(ones_r[:], 1.0)
            bc = ps5.tile([128, RPC], f32, tag="bc")
            nc.tensor.matmul(bc[:], ones_r[:], iz[:])
            izb = p5.tile([128, RPC], f32, tag="izb")
            nc.vector.tensor_copy(izb[:], bc[:])
            knn_nb = p5.tile([128, DT, RPC], bf16, tag="knnb")
            for d in range(DT):
                nc.vector.tensor_tensor(knn_s[:, d, :], knn_s[:, d, :], izb[:],
                                        op=AOT.mult)
                nc.vector.tensor_copy(knn_nb[:, d, :], knn_s[:, d, :])
            # wproj + bias
            wp = p5.tile([128, DT, D], bf16, tag="wp")
            bpr = p5.tile([128, DT], f32, tag="bpr")
            for d in range(DT):
                tw = p5b.tile([128, D], f32, tag="wpld")
                nc.sync.dma_start(tw[:], WPT[d * 128:(d + 1) * 128, :])
                nc.vector.tensor_copy(wp[:, d, :], tw[:])
                nc.sync.dma_start(bpr[:, d:d + 1],
                                  BPROJ[d * 128:(d + 1) * 128, :])
            knn_oT = p5.tile([128, DT, RPC], f32, tag="knoT")
            knn_oTb = p5.tile([128, DT, RPC], bf16, tag="knoTb")
            for d in range(DT):
                ps = ps5.tile([128, RPC], f32, tag="pmm")
                for e in range(DT):
                    nc.tensor.matmul(ps[:], wp[:, e, d * 128:(d + 1) * 128],
                                     knn_nb[:, e, :], start=(e == 0),
                                     stop=(e == DT - 1))
                nc.vector.tensor_scalar(knn_oT[:, d, :], ps[:],
                                        bpr[:, d:d + 1], scalar2=None,
                                        op0=AOT.add)
                nc.vector.tensor_copy(knn_oTb[:, d, :], knn_oT[:, d, :])
            # gate
            wg = p5.tile([128, 2 * DT], bf16, tag="wg")
            wgf = p5.tile([128, 2 * DT], f32, tag="wgf")
            nc.sync.dma_start(wgf[:],
                              WGT[:].rearrange("(a p) 1 -> p a", p=128))
            nc.vector.tensor_copy(wg[:], wgf[:])
            bgt = p5.tile([1, 1], f32, tag="bgt")
            nc.sync.dma_start(bgt[:], BG[:])
            gps = ps5.tile([1, RPC], f32, tag="gps")
            for e in range(DT):
                nc.tensor.matmul(gps[:], wg[:, e:e + 1], attn_oTb[:, e, :],
                                 start=(e == 0), stop=False)
            for e in range(DT):
                nc.tensor.matmul(gps[:], wg[:, DT + e:DT + e + 1],
                                 knn_oTb[:, e, :], start=False,
                                 stop=(e == DT - 1))
            gate = p5.tile([1, RPC], f32, tag="gate")
            nc.scalar.activation(gate[:], gps[:], AF.Sigmoid, bias=bgt[0:1, 0:1])
            gbc = ps5.tile([128, RPC], f32, tag="bc")
            nc.tensor.matmul(gbc[:], ones_r[:], gate[:])
            gateb = p5.tile([128, RPC], f32, tag="gateb")
            nc.vector.tensor_copy(gateb[:], gbc[:])
            # combine + transpose out
            for d in range(DT):
                dif = p5b.tile([128, RPC], f32, tag="dif")
                nc.vector.tensor_tensor(dif[:], attn_oT[:, d, :],
                                        knn_oT[:, d, :], op=AOT.subtract)
                nc.vector.tensor_tensor(dif[:], dif[:], gateb[:], op=AOT.mult)
                nc.vector.tensor_tensor(knn_oT[:, d, :], knn_oT[:, d, :],
                                        dif[:], op=AOT.add)
            for m in range(MT):
                orow = p5b.tile([128, D], f32, tag="orow")
                for d in range(DT):
                    tp = ps5t.tile([128, 128], f32, tag="tp")
                    nc.tensor.transpose(tp[:],
                                        knn_oT[:, d, m * 128:(m + 1) * 128],
                                        identf[:])
                    nc.vector.tensor_copy(orow[:, d * 128:(d + 1) * 128], tp[:])
                nc.sync.dma_start(OUT[m * 128:(m + 1) * 128, :], orow[:])

    nc.compile()
    _BUILT["nc"] = nc
    return nc


def kernel(x, store_keys, store_vals, Wq, Wk, Wv, Wo, Wkk, Wproj, bproj,
           Wg, bg):
    import os
    x = np.asarray(x, np.float32)
    store_keys = np.asarray(store_keys, np.float32)
    store_vals = np.asarray(store_vals, np.float32)
    Wq, Wk, Wv, Wo, Wkk, Wproj = (np.asarray(w, np.float32)
                                  for w in (Wq, Wk, Wv, Wo, Wkk, Wproj))
    bproj = np.asarray(bproj, np.float32)
    Wg = np.asarray(Wg, np.float32)
    bg = np.asarray(bg, np.float32)

    nc = _build()

    xtb = [np.ascontiguousarray(x[b].T) for b in range(B)]   # [D, S] each
    wqt = np.ascontiguousarray(Wq.T)
    wkt = np.ascontiguousarray(Wk.T)
    wvt = np.ascontiguousarray(Wv.T)
    wot = np.ascontiguousarray(Wo.T)
    wkkt = np.ascontiguousarray(Wkk.T)
    wpt = np.ascontiguousarray(Wproj.T)
    wgt = np.ascontiguousarray(Wg[0].reshape(2 * D, 1))
    bgt = bg.reshape(1, 1)
    bprojc = np.ascontiguousarray(bproj.reshape(D, 1))

    in_maps = []
    for c in range(NCORES):
        b, blk = c // 4, c % 4
        q0 = blk * 256
        validn = np.zeros((128, 3), np.float32)
        for ch in range(3):
            if (ch + 1) * 256 > q0:
                validn[:, ch] = NEG
        in_maps.append({
            "xt": xtb[b],
            "xtq": np.ascontiguousarray(xtb[b][:, q0:q0 + RPC]),
            "keys": store_keys[c * SS:(c + 1) * SS],
            "vals": store_vals[c * SS:(c + 1) * SS],
            "wqt": wqt, "wkt": wkt, "wvt": wvt, "wot": wot,
            "wkkt": wkkt, "wpt": wpt, "wgt": wgt, "bg": bgt,
            "bproj": bprojc, "validn": validn,
        })

    from concourse.bass_utils import run_bass_kernel_spmd
    trace = bool(os.environ.get("KNN_TRACE"))
    res = run_bass_kernel_spmd(nc, in_maps, list(range(NCORES)), trace=trace,
                               tmpdir=os.environ.get("KNN_TRACE_DIR"))
    if trace:
        _BUILT["exec_time_ns"] = res.exec_time_ns
    out = np.concatenate([res.results[c]["out"] for c in range(NCORES)], axis=0)
    return out.reshape(B, S, D).astype(np.float32)


# revision 9
# speedup vs baseline: 1.0393x; 1.0393x over previous
"""KNN-attention layer on 8 Trainium2 NeuronCores (Bass/Tile).

Sharding: core c owns query rows [256c, 256c+256) (batch c//4) and store
shard [4096c, 4096(c+1)).

Per-core program (static, identical on all cores):
  P0  knn-query projection (bf16 hi/lo split-3 matmul for fp32-exact sims),
      normalize, transpose, AllGather qn across cores; build diag causal masks.
  P1  per store half (2048): normalize+split+transpose keys; sims = qn @ kn.T
      for all 2048 rows via split-3; spill sims to DRAM; local top-32 per half
      (max8 + match_replace); in half 1, merge halves and AllGather candidates.
  P2  global threshold t (32nd) and max m per row from gathered candidates.
  P3  per half: reload sims, W = (s >= t) * exp((s-m)/T) in bf16 (+Z accum),
      transpose W, weighted matmul with store_vals -> unnormalized knn partial
      [1024, 2048]; pack + ReduceScatter (sum over cores, scatter by row block).
  P4  causal self-attention for own row block (bf16), static chunking:
      3 absolute 256-wide key pre-chunks gated by a per-core validity column
      + 1 diagonal chunk from the own-block x slice. Overlaps the RS (no
      gpsimd ops here; collectives own the gpsimd queue).
  P5  normalize knn by Z, project (+bias), gated combine, transpose to
      row-major, output [256, 1024] per core.

Selection precision: sims are computed as q_hi@k_hi + q_hi@k_lo + q_lo@k_hi
(bf16 splits, fp32 PSUM accumulation) which matches fp32 sims to ~1e-7 —
required because the global top-32 set must match the fp32 reference's.
"""
import sys

sys.path.insert(0, "/opt/trn_rl_repo")

import numpy as np

# ---------------- problem constants ----------------
B, S, D = 2, 1024, 1024
H, HD = 16, 64
N = 32768
KNN_K = 32
TEMP = 0.1
NCORES = 8
ROWS = B * S                # 2048
RPC = ROWS // NCORES        # 256 rows per core
SS = N // NCORES            # 4096 stores per core
HNS = SS // 2               # 2048 per half
DT = D // 128               # 8 feature tiles
RT = ROWS // 128            # 16 global row tiles
MT = RPC // 128             # 2 own row tiles
HIT = HNS // 128            # 16 store tiles per half
NEG = -1e30
RG = [list(range(NCORES))]

_BUILT = {}


def _build():
    if "nc" in _BUILT:
        return _BUILT["nc"]
    from contextlib import ExitStack
    import concourse.bass as bass
    import concourse.tile as tile
    from concourse import bacc, mybir
    from concourse.masks import make_identity

    f32 = mybir.dt.float32
    bf16 = mybir.dt.bfloat16
    AOT = mybir.AluOpType
    AF = mybir.ActivationFunctionType

    nc = bacc.Bacc("TRN2", target_bir_lowering=False, debug=False,
                   num_devices=NCORES)

    # ---- I/O ----
    XT = nc.dram_tensor("xt", [D, S], f32, kind="ExternalInput").ap()
    XTQ = nc.dram_tensor("xtq", [D, RPC], f32, kind="ExternalInput").ap()
    KEYS = nc.dram_tensor("keys", [SS, D], f32, kind="ExternalInput").ap()
    VALS = nc.dram_tensor("vals", [SS, D], f32, kind="ExternalInput").ap()
    WQT = nc.dram_tensor("wqt", [D, D], f32, kind="ExternalInput").ap()
    WKT = nc.dram_tensor("wkt", [D, D], f32, kind="ExternalInput").ap()
    WVT = nc.dram_tensor("wvt", [D, D], f32, kind="ExternalInput").ap()
    WOT = nc.dram_tensor("wot", [D, D], f32, kind="ExternalInput").ap()
    WKKT = nc.dram_tensor("wkkt", [D, D], f32, kind="ExternalInput").ap()
    WPT = nc.dram_tensor("wpt", [D, D], f32, kind="ExternalInput").ap()
    WGT = nc.dram_tensor("wgt", [2 * D, 1], f32, kind="ExternalInput").ap()
    BG = nc.dram_tensor("bg", [1, 1], f32, kind="ExternalInput").ap()
    BPROJ = nc.dram_tensor("bproj", [D, 1], f32, kind="ExternalInput").ap()
    VALIDN = nc.dram_tensor("validn", [128, 3], f32, kind="ExternalInput").ap()
    OUT = nc.dram_tensor("out", [RPC, D], f32, kind="ExternalOutput").ap()

    # ---- DRAM scratch ----
    simsbuf = nc.dram_tensor("simsbuf", [RT, 2, 128, HNS], f32,
                             kind="Internal").ap()
    qn_b = nc.dram_tensor("qn_b", [2, D, RPC], bf16, kind="Internal").ap()
    qn_g = nc.dram_tensor("qn_g", [NCORES, 2, D, RPC], bf16, kind="Internal",
                          addr_space="Shared").ap()
    cand_b = nc.dram_tensor("cand_b", [RT, 128, 24], f32, kind="Internal").ap()
    cand_g = nc.dram_tensor("cand_g", [NCORES, RT, 128, 24], f32,
                            kind="Internal", addr_space="Shared").ap()
    rs_in = nc.dram_tensor("rs_in", [NCORES, 1152, RPC], f32,
                           kind="Internal").ap()
    rs_out = nc.dram_tensor("rs_out", [1152, RPC], f32, kind="Internal").ap()

    def rsqrt_newton(pool, n2, tagp):
        r0 = pool.tile([128, 1], f32, tag=tagp + "r0", name=tagp + "r0")
        nc.vector.reciprocal(r0[:], n2)
        y = pool.tile([128, 1], f32, tag=tagp + "y", name=tagp + "y")
        nc.scalar.activation(y[:], r0[:], AF.Sqrt)
        for it in ("a", "b"):
            yy = pool.tile([128, 1], f32, tag=tagp + it + "1",
                           name=tagp + it + "1")
            nc.vector.tensor_tensor(yy[:], y[:], y[:], op=AOT.mult)
            nc.vector.tensor_tensor(yy[:], n2, yy[:], op=AOT.mult)
            nc.vector.tensor_scalar(yy[:], yy[:], -0.5, scalar2=1.5,
                                    op0=AOT.mult, op1=AOT.add)
            y2 = pool.tile([128, 1], f32, tag=tagp + it + "2",
                           name=tagp + it + "2")
            nc.vector.tensor_tensor(y2[:], y[:], yy[:], op=AOT.mult)
            y = y2
        return y

    def split_hi_lo(src, hi, lo):
        nc.vector.tensor_copy(hi, src)
        nc.vector.tensor_tensor(lo, src, hi, op=AOT.subtract)

    with tile.TileContext(nc) as tc, ExitStack() as ctx:
        res = ctx.enter_context(tc.tile_pool(name="res", bufs=1))
        ident = res.tile([128, 128], bf16)
        make_identity(nc, ident)
        identf = res.tile([128, 128], f32)
        make_identity(nc, identf)
        maxes_all = res.tile([128, RT, 48], f32)
        t_m = res.tile([128, RT, 2], f32)
        attn_oT = res.tile([128, DT, RPC], f32)
        attn_oTb = res.tile([128, DT, RPC], bf16)
        dmask = res.tile([128, MT, 256], f32)

        # ================= P0: qn + AllGather + diag masks ================
        with tc.tile_pool(name="p0", bufs=2) as p0, \
             tc.tile_pool(name="ps0", bufs=2, space="PSUM") as ps0, \
             tc.tile_pool(name="ps0t", bufs=2, space="PSUM") as ps0t:
            # diagonal causal masks (additive 0/NEG), before any collective
            nc.vector.memset(dmask[:], 0.0)
            for m in range(MT):
                nc.gpsimd.affine_select(
                    dmask[:, m, :], dmask[:, m, :],
                    pattern=[[-1, 256]], compare_op=AOT.is_ge, fill=NEG,
                    base=m * 128, channel_multiplier=1)
            xq_hi = p0.tile([128, DT, RPC], bf16, tag="xqh")
            xq_lo = p0.tile([128, DT, RPC], bf16, tag="xql")
            wk_hi = p0.tile([128, DT, D], bf16, tag="wkh")
            wk_lo = p0.tile([128, DT, D], bf16, tag="wkl")
            for d in range(DT):
                t = p0.tile([128, RPC], f32, tag="ld")
                nc.sync.dma_start(t[:], XTQ[d * 128:(d + 1) * 128, :])
                split_hi_lo(t[:], xq_hi[:, d, :], xq_lo[:, d, :])
                tw = p0.tile([128, D], f32, tag="ldw")
                nc.sync.dma_start(tw[:], WKKT[d * 128:(d + 1) * 128, :])
                split_hi_lo(tw[:], wk_hi[:, d, :], wk_lo[:, d, :])
            for m in range(MT):
                qrow = p0.tile([128, D], f32, tag="qrow")
                for chn in range(2):
                    ps = ps0.tile([128, 512], f32, tag="mm")
                    cs = slice(chn * 512, (chn + 1) * 512)
                    ms = slice(m * 128, (m + 1) * 128)
                    for d in range(DT):
                        nc.tensor.matmul(ps[:], xq_hi[:, d, ms], wk_hi[:, d, cs],
                                         start=(d == 0), stop=False)
                        nc.tensor.matmul(ps[:], xq_hi[:, d, ms], wk_lo[:, d, cs],
                                         start=False, stop=False)
                        nc.tensor.matmul(ps[:], xq_lo[:, d, ms], wk_hi[:, d, cs],
                                         start=False, stop=(d == DT - 1))
                    nc.vector.tensor_copy(qrow[:, cs], ps[:])
                n2 = p0.tile([128, 1], f32, tag="n2")
                sq = p0.tile([128, D], f32, tag="sq")
                nc.scalar.activation(sq[:], qrow[:], AF.Square, accum_out=n2[:])
                inv = rsqrt_newton(p0, n2[:], "nq")
                nc.vector.tensor_scalar_mul(qrow[:], qrow[:], inv[:])
                q_hi = p0.tile([128, D], bf16, tag="qhi")
                q_lo = p0.tile([128, D], bf16, tag="qlo")
                split_hi_lo(qrow[:], q_hi[:], q_lo[:])
                for d in range(DT):
                    for hl, src in enumerate((q_hi, q_lo)):
                        tp = ps0t.tile([128, 128], bf16, tag="tp")
                        nc.tensor.transpose(tp[:], src[:, d * 128:(d + 1) * 128],
                                            ident[:])
                        sb = p0.tile([128, 128], bf16, tag="tpo")
                        nc.vector.tensor_copy(sb[:], tp[:])
                        nc.sync.dma_start(
                            qn_b[hl, d * 128:(d + 1) * 128,
                                 m * 128:(m + 1) * 128], sb[:])
            nc.gpsimd.collective_compute(
                "AllGather", AOT.bypass, replica_groups=RG,
                ins=[qn_b.opt()], outs=[qn_g.opt()])

        # ============ P1: keys + sims + local topk (both halves) ==========
        def keys_half(h, p1, p1b, p1c, ps1t, knT_hi, knT_lo):
            for i in range(HIT):
                kt = p1b.tile([128, D], f32, tag="kld", name="kld")
                nc.sync.dma_start(
                    kt[:], KEYS[h * HNS + i * 128:h * HNS + (i + 1) * 128, :])
                n2 = p1c.tile([128, 1], f32, tag="kn2", name="kn2")
                sq = p1c.tile([128, D], f32, tag="ksq", name="ksq")
                nc.scalar.activation(sq[:], kt[:], AF.Square, accum_out=n2[:])
                inv = rsqrt_newton(p1c, n2[:], "nk")
                nc.vector.tensor_scalar_mul(kt[:], kt[:], inv[:])
                k_hi = p1b.tile([128, D], bf16, tag="khi", name="khi")
                k_lo = p1b.tile([128, D], bf16, tag="klo", name="klo")
                split_hi_lo(kt[:], k_hi[:], k_lo[:])
                for d in range(DT):
                    for src, dst in ((k_hi, knT_hi), (k_lo, knT_lo)):
                        tp = ps1t.tile([128, 128], bf16, tag="tp", name="tp")
                        nc.tensor.transpose(
                            tp[:], src[:, d * 128:(d + 1) * 128], ident[:])
                        if d % 2 == 0:
                            nc.scalar.activation(
                                dst[:, d, i * 128:(i + 1) * 128], tp[:], AF.Copy)
                        else:
                            nc.vector.tensor_copy(
                                dst[:, d, i * 128:(i + 1) * 128], tp[:])

        def sims_half(h, p1b, p1c, ps1, knT_hi, knT_lo, merge_local):
            for rt in range(RT):
                cb, ml = rt // 2, rt % 2
                qh = p1b.tile([128, DT, 128], bf16, tag="qsh", name="qsh")
                ql = p1b.tile([128, DT, 128], bf16, tag="qsl", name="qsl")
                for hl, dst in ((0, qh), (1, ql)):
                    nc.sync.dma_start(
                        dst[:],
                        qn_g[cb, hl, :, ml * 128:(ml + 1) * 128].rearrange(
                            "(dt p) c -> p dt c", p=128))
                sims = p1b.tile([128, HNS], f32, tag="sims", name="sims")
                pss = [ps1.tile([128, 512], f32, tag=f"mm{i}", name=f"mm{i}")
                       for i in range(4)]
                for d in range(DT):
                    for chn in range(4):
                        nc.tensor.matmul(pss[chn][:], qh[:, d, :],
                                         knT_hi[:, d, chn * 512:(chn + 1) * 512],
                                         start=(d == 0), stop=False)
                    for chn in range(4):
                        nc.tensor.matmul(pss[chn][:], qh[:, d, :],
                                         knT_lo[:, d, chn * 512:(chn + 1) * 512],
                                         start=False, stop=False)
                    for chn in range(4):
                        nc.tensor.matmul(pss[chn][:], ql[:, d, :],
                                         knT_hi[:, d, chn * 512:(chn + 1) * 512],
                                         start=False, stop=(d == DT - 1))
                for chn in range(4):
                    nc.scalar.activation(sims[:, chn * 512:(chn + 1) * 512],
                                         pss[chn][:], AF.Copy)
                nc.sync.dma_start(simsbuf[rt, h], sims[:])
                scr = p1c.tile([128, HNS], f32, tag="scr", name="scr")
                nc.vector.tensor_copy(scr[:], sims[:])
                for r in range(3):
                    mx = maxes_all[:, rt, h * 24 + r * 8:h * 24 + (r + 1) * 8]
                    nc.vector.max(mx, scr[:])
                    nc.vector.match_replace(scr[:], mx, scr[:], NEG)
                if merge_local:
                    mscr = p1c.tile([128, 48], f32, tag="mscr", name="mscr")
                    nc.vector.tensor_copy(mscr[:], maxes_all[:, rt, :])
                    loc = p1c.tile([128, 24], f32, tag="loc", name="loc")
                    for r in range(3):
                        nc.vector.max(loc[:, r * 8:(r + 1) * 8], mscr[:])
                        nc.vector.match_replace(mscr[:], loc[:, r * 8:(r + 1) * 8],
                                                mscr[:], NEG)
                    nc.sync.dma_start(cand_b[rt], loc[:])

        with tc.tile_pool(name="vp", bufs=2) as vpool:
            vals_bf0 = vpool.tile([128, HIT, D], bf16, tag="vals", name="vals0")
            for i in range(HIT):
                vt = vpool.tile([128, D], f32, tag="vld", name="vld")
                nc.sync.dma_start(vt[:], VALS[i * 128:(i + 1) * 128, :])
                nc.vector.tensor_copy(vals_bf0[:, i, :], vt[:])
            for h in range(2):
                with tc.tile_pool(name=f"p1_{h}", bufs=1) as p1, \
                     tc.tile_pool(name=f"p1b_{h}", bufs=2) as p1b, \
                     tc.tile_pool(name=f"p1c_{h}", bufs=1) as p1c, \
                     tc.tile_pool(name=f"ps1_{h}", bufs=1, space="PSUM") as ps1, \
                     tc.tile_pool(name=f"ps1t_{h}", bufs=2, space="PSUM") as ps1t:
                    knT_hi = p1.tile([128, DT, HNS], bf16, tag="knh", name="knh")
                    knT_lo = p1.tile([128, DT, HNS], bf16, tag="knl", name="knl")
                    keys_half(h, p1, p1b, p1c, ps1t, knT_hi, knT_lo)
                    sims_half(h, p1b, p1c, ps1, knT_hi, knT_lo, h == 1)
            nc.gpsimd.collective_compute(
                "AllGather", AOT.bypass, replica_groups=RG,
                ins=[cand_b.opt()], outs=[cand_g.opt()])

            # ============ P2: global threshold/max ============
            with tc.tile_pool(name="p2", bufs=2) as p2:
                for rt in range(RT):
                    gall = p2.tile([128, NCORES * 24], f32, tag="gall")
                    for cb in range(NCORES):
                        nc.sync.dma_start(gall[:, cb * 24:(cb + 1) * 24],
                                          cand_g[cb, rt])
                    gm = p2.tile([128, 4, 8], f32, tag="gm")
                    for r in range(4):
                        nc.vector.max(gm[:, r, :], gall[:])
                        nc.vector.match_replace(gall[:], gm[:, r, :], gall[:], NEG)
                    nc.vector.tensor_copy(t_m[:, rt, 0:1], gm[:, 3, 7:8])
                    nc.vector.tensor_copy(t_m[:, rt, 1:2], gm[:, 0, 0:1])

            # ============ P3: W, weighted matmul, ReduceScatter ============
            with tc.tile_pool(name="p3", bufs=1) as p3, \
                 tc.tile_pool(name="p3b", bufs=2) as p3b, \
                 tc.tile_pool(name="ps3", bufs=4, space="PSUM") as ps3, \
                 tc.tile_pool(name="ps3t", bufs=2, space="PSUM") as ps3t:
                unorm = p3.tile([128, DT, ROWS], f32, tag="unorm")
                z_cols = p3.tile([128, RT], f32, tag="zc")
                for h in range(2):
                    if h == 0:
                        vals_bf = vals_bf0
                    else:
                        vals_bf = vpool.tile([128, HIT, D], bf16, tag="vals",
                                             name="vals1")
                        for i in range(HIT):
                            vt = vpool.tile([128, D], f32, tag="vld", name="vld")
                            nc.sync.dma_start(
                                vt[:],
                                VALS[HNS + i * 128:HNS + (i + 1) * 128, :])
                            nc.vector.tensor_copy(vals_bf[:, i, :], vt[:])
                    for g in range(4):
                        w_T = p3.tile([128, HIT, 512], bf16, tag="wT",
                                      name="wT")
                        for rl in range(4):
                            rt = g * 4 + rl
                            sims = p3.tile([128, HNS], f32, tag="srl",
                                           name="srl")
                            nc.sync.dma_start(sims[:], simsbuf[rt, h])
                            mbias = p3b.tile([128, 1], f32, tag="mb", name="mb")
                            nc.vector.tensor_scalar_mul(mbias[:],
                                                        t_m[:, rt, 1:2],
                                                        -1.0 / TEMP)
                            expw = p3b.tile([128, HNS], f32, tag="expw",
                                            name="expw")
                            nc.scalar.activation(expw[:], sims[:], AF.Exp,
                                                 bias=mbias[:], scale=1.0 / TEMP)
                            wmask = p3b.tile([128, HNS], bf16, tag="wm",
                                             name="wm")
                            z = p3b.tile([128, 1], f32, tag="z", name="z")
                            nc.vector.scalar_tensor_tensor(
                                wmask[:], sims[:], t_m[:, rt, 0:1], expw[:],
                                op0=AOT.is_ge, op1=AOT.mult, accum_out=z[:])
                            if h == 0:
                                nc.vector.tensor_copy(z_cols[:, rt:rt + 1], z[:])
                            else:
                                nc.vector.tensor_tensor(
                                    z_cols[:, rt:rt + 1], z_cols[:, rt:rt + 1],
                                    z[:], op=AOT.add)
                            for i in range(HIT):
                                tp = ps3t.tile([128, 128], bf16, tag="tp",
                                               name="tp")
                                nc.tensor.transpose(
                                    tp[:], wmask[:, i * 128:(i + 1) * 128],
                                    ident[:])
                                nc.vector.tensor_copy(
                                    w_T[:, i, rl * 128:(rl + 1) * 128], tp[:])
                        for d in range(DT):
                            ps = ps3.tile([128, 512], f32, tag="mm", name="mm")
                            for i in range(HIT):
                                nc.tensor.matmul(
                                    ps[:], vals_bf[:, i, d * 128:(d + 1) * 128],
                                    w_T[:, i, :], start=(i == 0),
                                    stop=(i == HIT - 1))
                            gs = slice(g * 512, (g + 1) * 512)
                            if h == 0:
                                nc.scalar.activation(unorm[:, d, gs], ps[:],
                                                     AF.Copy)
                            else:
                                nc.vector.tensor_tensor(unorm[:, d, gs],
                                                        unorm[:, d, gs], ps[:],
                                                        op=AOT.add)
                # pack rs_in
                for cb in range(NCORES):
                    for d in range(DT):
                        nc.sync.dma_start(
                            rs_in[cb, d * 128:(d + 1) * 128, :],
                            unorm[:, d, cb * RPC:(cb + 1) * RPC])
                zt = ps3t.tile([RT, 128], f32, tag="zt")
                nc.tensor.transpose(zt[:], z_cols[:], identf[:])
                zrow = p3.tile([RT, 128], f32, tag="zrow")
                nc.vector.tensor_copy(zrow[:], zt[:])
                zero = p3.tile([128, RPC], f32, tag="zero")
                nc.vector.memset(zero[:], 0.0)
                for cb in range(NCORES):
                    nc.sync.dma_start(
                        rs_in[cb, 1024:1025, :].rearrange(
                            "o (a b) -> (o a) b", a=2),
                        zrow[cb * 2:cb * 2 + 2, :])
                    nc.sync.dma_start(rs_in[cb, 1025:1152, :], zero[0:127, :])
                nc.gpsimd.collective_compute(
                    "ReduceScatter", AOT.add, replica_groups=RG,
                    ins=[rs_in.opt()], outs=[rs_out.opt()])

        # ================= P4: causal attention (own block) ===============
        with tc.tile_pool(name="p4", bufs=1) as p4, \
             tc.tile_pool(name="p4b", bufs=2) as p4b, \
             tc.tile_pool(name="ps4", bufs=2, space="PSUM") as ps4, \
             tc.tile_pool(name="ps4s", bufs=2, space="PSUM") as ps4s, \
             tc.tile_pool(name="ps4p", bufs=2, space="PSUM") as ps4p, \
             tc.tile_pool(name="ps4t", bufs=2, space="PSUM") as ps4t:
            xb = p4.tile([128, DT, S], bf16, tag="xb")
            xqb = p4.tile([128, DT, RPC], bf16, tag="xqb")
            wq = p4.tile([128, DT, D], bf16, tag="wq")
            wk = p4.tile([128, DT, D], bf16, tag="wk")
            wv = p4.tile([128, DT, D], bf16, tag="wv")
            wo = p4.tile([128, DT, D], bf16, tag="wo")
            vneg = p4.tile([128, 3], f32, tag="vneg")
            nc.sync.dma_start(vneg[:], VALIDN[:])
            for d in range(DT):
                t = p4b.tile([128, S], f32, tag="xld")
                nc.sync.dma_start(t[:], XT[d * 128:(d + 1) * 128, :])
                nc.vector.tensor_copy(xb[:, d, :], t[:])
                tq = p4b.tile([128, RPC], f32, tag="xqld")
                nc.sync.dma_start(tq[:], XTQ[d * 128:(d + 1) * 128, :])
                nc.vector.tensor_copy(xqb[:, d, :], tq[:])
                for src, dst in ((WQT, wq), (WKT, wk), (WVT, wv), (WOT, wo)):
                    tw = p4b.tile([128, D], f32, tag="wld")
                    nc.sync.dma_start(tw[:], src[d * 128:(d + 1) * 128, :])
                    nc.vector.tensor_copy(dst[:, d, :], tw[:])
            qT = p4.tile([128, DT, RPC], bf16, tag="qT")
            kTp = p4.tile([128, DT, 768], bf16, tag="kTp")
            kTd = p4.tile([128, DT, RPC], bf16, tag="kTd")
            for d in range(DT):
                ps = ps4.tile([128, 512], f32, tag="pj")
                nc_d = slice(d * 128, (d + 1) * 128)
                for e in range(DT):
                    nc.tensor.matmul(ps[:, :RPC], wq[:, e, nc_d], xqb[:, e, :],
                                     start=(e == 0), stop=(e == DT - 1))
                nc.vector.tensor_copy(qT[:, d, :], ps[:, :RPC])
                for e in range(DT):
                    nc.tensor.matmul(ps[:, :RPC], wk[:, e, nc_d], xqb[:, e, :],
                                     start=(e == 0), stop=(e == DT - 1))
                nc.vector.tensor_copy(kTd[:, d, :], ps[:, :RPC])
                for chn in range(2):
                    cs = slice(chn * 512, min((chn + 1) * 512, 768))
                    w_ = cs.stop - cs.start
                    for e in range(DT):
                        nc.tensor.matmul(ps[:, :w_], wk[:, e, nc_d],
                                         xb[:, e, cs], start=(e == 0),
                                         stop=(e == DT - 1))
                    nc.vector.tensor_copy(kTp[:, d, cs], ps[:, :w_])
            vR = p4.tile([128, 6, D], bf16, tag="vR")
            vRd = p4.tile([128, MT, D], bf16, tag="vRd")
            for k in range(6):
                for chn in range(2):
                    ps = ps4.tile([128, 512], f32, tag="pj")
                    cs = slice(chn * 512, (chn + 1) * 512)
                    for e in range(DT):
                        nc.tensor.matmul(ps[:], xb[:, e, k * 128:(k + 1) * 128],
                                         wv[:, e, cs], start=(e == 0),
                                         stop=(e == DT - 1))
                    nc.vector.tensor_copy(vR[:, k, cs], ps[:])
            for m in range(MT):
                for chn in range(2):
                    ps = ps4.tile([128, 512], f32, tag="pj")
                    cs = slice(chn * 512, (chn + 1) * 512)
                    for e in range(DT):
                        nc.tensor.matmul(ps[:], xqb[:, e, m * 128:(m + 1) * 128],
                                         wv[:, e, cs], start=(e == 0),
                                         stop=(e == DT - 1))
                    nc.vector.tensor_copy(vRd[:, m, cs], ps[:])
            acT = p4.tile([128, DT, RPC], bf16, tag="acT")
            for hh in range(H):
                d, half = hh // 2, hh % 2
                hp = slice(64 * half, 64 * half + 64)
                for m in range(MT):
                    sc = p4b.tile([128, 1024], f32, tag="sc")
                    for chn in range(3):
                        ps = ps4s.tile([128, 256], f32, tag="sps")
                        nc.tensor.matmul(ps[:], qT[hp, d, m * 128:(m + 1) * 128],
                                         kTp[hp, d, chn * 256:(chn + 1) * 256])
                        nc.vector.tensor_scalar(
                            sc[:, chn * 256:(chn + 1) * 256], ps[:],
                            vneg[:, chn:chn + 1], scalar2=None, op0=AOT.add)
                    ps = ps4s.tile([128, 256], f32, tag="sps")
                    nc.tensor.matmul(ps[:], qT[hp, d, m * 128:(m + 1) * 128],
                                     kTd[hp, d, :])
                    nc.vector.tensor_tensor(sc[:, 768:1024], ps[:],
                                            dmask[:, m, :], op=AOT.add)
                    rm = p4b.tile([128, 1], f32, tag="rm")
                    nc.vector.tensor_reduce(rm[:], sc[:],
                                            axis=mybir.AxisListType.XYZW,
                                            op=AOT.max)
                    nc.vector.tensor_scalar_mul(rm[:], rm[:], -0.125)
                    ex = p4b.tile([128, 1024], f32, tag="ex")
                    zr = p4b.tile([128, 1], f32, tag="zr")
                    nc.scalar.activation(ex[:], sc[:], AF.Exp, bias=rm[:],
                                         scale=0.125, accum_out=zr[:])
                    iz = p4b.tile([128, 1], f32, tag="iz")
                    nc.vector.reciprocal(iz[:], zr[:])
                    ab = p4b.tile([128, 1024], bf16, tag="ab")
                    nc.scalar.activation(ab[:], ex[:], AF.Copy, scale=iz[:])
                    pv = ps4p.tile([128, 128], f32, tag="pv")
                    for k in range(8):
                        tp = ps4t.tile([128, 128], bf16, tag="tp")
                        nc.tensor.transpose(tp[:], ab[:, k * 128:(k + 1) * 128],
                                            ident[:])
                        at = p4b.tile([128, 128], bf16, tag="at")
                        nc.vector.tensor_copy(at[:], tp[:])
                        vsrc = (vR[:, k, hh * 64:(hh + 1) * 64] if k < 6 else
                                vRd[:, k - 6, hh * 64:(hh + 1) * 64])
                        nc.tensor.matmul(pv[hp, :], vsrc, at[:],
                                         start=(k == 0), stop=(k == 7))
                    nc.vector.tensor_copy(acT[hp, d, m * 128:(m + 1) * 128],
                                          pv[hp, :])
            for d in range(DT):
                ps = ps4.tile([128, 512], f32, tag="pj")
                for e in range(DT):
                    nc.tensor.matmul(ps[:, :RPC], wo[:, e, d * 128:(d + 1) * 128],
                                     acT[:, e, :], start=(e == 0),
                                     stop=(e == DT - 1))
                nc.vector.tensor_copy(attn_oT[:, d, :], ps[:, :RPC])
                nc.vector.tensor_copy(attn_oTb[:, d, :], ps[:, :RPC])

        # ================= P5: finale =================
        with tc.tile_pool(name="p5", bufs=1) as p5, \
             tc.tile_pool(name="p5b", bufs=2) as p5b, \
             tc.tile_pool(name="ps5", bufs=2, space="PSUM") as ps5, \
             tc.tile_pool(name="ps5t", bufs=2, space="PSUM") as ps5t:
            knn_s = p5.tile([128, DT, RPC], f32, tag="kns")
            for d in range(DT):
                nc.sync.dma_start(knn_s[:, d, :],
                                  rs_out[d * 128:(d + 1) * 128, :])
            zg = p5.tile([1, RPC], f32, tag="zg")
            nc.sync.dma_start(zg[:], rs_out[1024:1025, :])
            iz = p5.tile([1, RPC], f32, tag="izg")
            nc.vector.reciprocal(iz[:], zg[:])
            ones_r = p5.tile([1, 128], f32, tag="ones")
            nc.vector.memset(ones_r[:], 1.0)
            bc = ps5.tile([128, RPC], f32, tag="bc")
            nc.tensor.matmul(bc[:], ones_r[:], iz[:])
            izb = p5.tile([128, RPC], f32, tag="izb")
            nc.vector.tensor_copy(izb[:], bc[:])
            knn_nb = p5.tile([128, DT, RPC], bf16, tag="knnb")
            for d in range(DT):
                nc.vector.tensor_tensor(knn_s[:, d, :], knn_s[:, d, :], izb[:],
                                        op=AOT.mult)
                nc.vector.tensor_copy(knn_nb[:, d, :], knn_s[:, d, :])
            wp = p5.tile([128, DT, D], bf16, tag="wp")
            bpr = p5.tile([128, DT], f32, tag="bpr")
            for d in range(DT):
                tw = p5b.tile([128, D], f32, tag="wpld")
                nc.sync.dma_start(tw[:], WPT[d * 128:(d + 1) * 128, :])
                nc.vector.tensor_copy(wp[:, d, :], tw[:])
                nc.sync.dma_start(bpr[:, d:d + 1],
                                  BPROJ[d * 128:(d + 1) * 128, :])
            knn_oT = p5.tile([128, DT, RPC], f32, tag="knoT")
            knn_oTb = p5.tile([128, DT, RPC], bf16, tag="knoTb")
            for d in range(DT):
                ps = ps5.tile([128, RPC], f32, tag="pmm")
                for e in range(DT):
                    nc.tensor.matmul(ps[:], wp[:, e, d * 128:(d + 1) * 128],
                                     knn_nb[:, e, :], start=(e == 0),
                                     stop=(e == DT - 1))
                nc.vector.tensor_scalar(knn_oT[:, d, :], ps[:],
                                        bpr[:, d:d + 1], scalar2=None,
                                        op0=AOT.add)
                nc.vector.tensor_copy(knn_oTb[:, d, :], knn_oT[:, d, :])
            wg = p5.tile([128, 2 * DT], bf16, tag="wg")
            wgf = p5.tile([128, 2 * DT], f32, tag="wgf")
            nc.sync.dma_start(wgf[:],
                              WGT[:].rearrange("(a p) 1 -> p a", p=128))
            nc.vector.tensor_copy(wg[:], wgf[:])
            bgt = p5.tile([1, 1], f32, tag="bgt")
            nc.sync.dma_start(bgt[:], BG[:])
            gps = ps5.tile([1, RPC], f32, tag="gps")
            for e in range(DT):
                nc.tensor.matmul(gps[:], wg[:, e:e + 1], attn_oTb[:, e, :],
                                 start=(e == 0), stop=False)
            for e in range(DT):
                nc.tensor.matmul(gps[:], wg[:, DT + e:DT + e + 1],
                                 knn_oTb[:, e, :], start=False,
                                 stop=(e == DT - 1))
            gate = p5.tile([1, RPC], f32, tag="gate")
            nc.scalar.activation(gate[:], gps[:], AF.Sigmoid, bias=bgt[0:1, 0:1])
            gbc = ps5.tile([128, RPC], f32, tag="bc")
            nc.tensor.matmul(gbc[:], ones_r[:], gate[:])
            gateb = p5.tile([128, RPC], f32, tag="gateb")
            nc.vector.tensor_copy(gateb[:], gbc[:])
            for d in range(DT):
                dif = p5b.tile([128, RPC], f32, tag="dif")
                nc.vector.tensor_tensor(dif[:], attn_oT[:, d, :],
                                        knn_oT[:, d, :], op=AOT.subtract)
                nc.vector.tensor_tensor(dif[:], dif[:], gateb[:], op=AOT.mult)
                nc.vector.tensor_tensor(knn_oT[:, d, :], knn_oT[:, d, :],
                                        dif[:], op=AOT.add)
            for m in range(MT):
                orow = p5b.tile([128, D], f32, tag="orow")
                for d in range(DT):
                    tp = ps5t.tile([128, 128], f32, tag="tp")
                    nc.tensor.transpose(tp[:],
                                        knn_oT[:, d, m * 128:(m + 1) * 128],
                                        identf[:])
                    nc.vector.tensor_copy(orow[:, d * 128:(d + 1) * 128], tp[:])
                nc.sync.dma_start(OUT[m * 128:(m + 1) * 128, :], orow[:])

    nc.compile()
    _BUILT["nc"] = nc
    return nc


def kernel(x, store_keys, store_vals, Wq, Wk, Wv, Wo, Wkk, Wproj, bproj,
           Wg, bg):
    import os
    x = np.asarray(x, np.float32)
    store_keys = np.asarray(store_keys, np.float32)
    store_vals = np.asarray(store_vals, np.float32)
    Wq, Wk, Wv, Wo, Wkk, Wproj = (np.asarray(w, np.float32)
                                  for w in (Wq, Wk, Wv, Wo, Wkk, Wproj))
    bproj = np.asarray(bproj, np.float32)
    Wg = np.asarray(Wg, np.float32)
    bg = np.asarray(bg, np.float32)

    nc = _build()

    xtb = [np.ascontiguousarray(x[b].T) for b in range(B)]
    wqt = np.ascontiguousarray(Wq.T)
    wkt = np.ascontiguousarray(Wk.T)
    wvt = np.ascontiguousarray(Wv.T)
    wot = np.ascontiguousarray(Wo.T)
    wkkt = np.ascontiguousarray(Wkk.T)
    wpt = np.ascontiguousarray(Wproj.T)
    wgt = np.ascontiguousarray(Wg[0].reshape(2 * D, 1))
    bgt = bg.reshape(1, 1)
    bprojc = np.ascontiguousarray(bproj.reshape(D, 1))

    in_maps = []
    for c in range(NCORES):
        b, blk = c // 4, c % 4
        q0 = blk * 256
        validn = np.zeros((128, 3), np.float32)
        for ch in range(3):
            if (ch + 1) * 256 > q0:
                validn[:, ch] = NEG
        in_maps.append({
            "xt": xtb[b],
            "xtq": np.ascontiguousarray(xtb[b][:, q0:q0 + RPC]),
            "keys": store_keys[c * SS:(c + 1) * SS],
            "vals": store_vals[c * SS:(c + 1) * SS],
            "wqt": wqt, "wkt": wkt, "wvt": wvt, "wot": wot,
            "wkkt": wkkt, "wpt": wpt, "wgt": wgt, "bg": bgt,
            "bproj": bprojc, "validn": validn,
        })

    from concourse.bass_utils import run_bass_kernel_spmd
    trace = bool(os.environ.get("KNN_TRACE"))
    res = run_bass_kernel_spmd(nc, in_maps, list(range(NCORES)), trace=trace,
                               tmpdir=os.environ.get("KNN_TRACE_DIR"))
    if trace:
        _BUILT["exec_time_ns"] = res.exec_time_ns
    out = np.concatenate([res.results[c]["out"] for c in range(NCORES)], axis=0)
    return out.reshape(B, S, D).astype(np.float32)


# revision 12
# speedup vs baseline: 1.0750x; 1.0343x over previous
"""KNN-attention layer on 8 Trainium2 NeuronCores (Bass/Tile).

Sharding: core c owns query rows [256c, 256c+256) (batch c//4) and store
shard [4096c, 4096(c+1)).

Per-core program (static, identical on all cores):
  P0  knn-query projection (bf16 hi/lo split-3 matmul for fp32-exact sims),
      normalize, transpose, AllGather qn across cores; build diag causal masks.
  P1  per store half (2048): normalize+split+transpose keys; sims = qn @ kn.T
      for all 2048 rows via split-3; spill sims to DRAM; local top-32 per half
      (max8 + match_replace); in half 1, merge halves and AllGather candidates.
  P2  global threshold t (32nd) and max m per row from gathered candidates.
  P3  per half: reload sims, W = (s >= t) * exp((s-m)/T) in bf16 (+Z accum),
      transpose W, weighted matmul with store_vals -> unnormalized knn partial
      [1024, 2048]; pack + ReduceScatter (sum over cores, scatter by row block).
  P4  causal self-attention for own row block (bf16), static chunking:
      3 absolute 256-wide key pre-chunks gated by a per-core validity column
      + 1 diagonal chunk from the own-block x slice. Overlaps the RS (no
      gpsimd ops here; collectives own the gpsimd queue).
  P5  normalize knn by Z, project (+bias), gated combine, transpose to
      row-major, output [256, 1024] per core.

Selection precision: sims are computed as q_hi@k_hi + q_hi@k_lo + q_lo@k_hi
(bf16 splits, fp32 PSUM accumulation) which matches fp32 sims to ~1e-7 —
required because the global top-32 set must match the fp32 reference's.
"""
import sys

sys.path.insert(0, "/opt/trn_rl_repo")

import numpy as np

# ---------------- problem constants ----------------
B, S, D = 2, 1024, 1024
H, HD = 16, 64
N = 32768
KNN_K = 32
TEMP = 0.1
NCORES = 8
ROWS = B * S                # 2048
RPC = ROWS // NCORES        # 256 rows per core
SS = N // NCORES            # 4096 stores per core
HNS = SS // 2               # 2048 per half
DT = D // 128               # 8 feature tiles
RT = ROWS // 128            # 16 global row tiles
MT = RPC // 128             # 2 own row tiles
HIT = HNS // 128            # 16 store tiles per half
NEG = -1e30
RG = [list(range(NCORES))]

_BUILT = {}


def _build():
    if "nc" in _BUILT:
        return _BUILT["nc"]
    from contextlib import ExitStack
    import concourse.bass as bass
    import concourse.tile as tile
    from concourse import bacc, mybir
    from concourse.masks import make_identity

    f32 = mybir.dt.float32
    bf16 = mybir.dt.bfloat16
    AOT = mybir.AluOpType
    AF = mybir.ActivationFunctionType

    nc = bacc.Bacc("TRN2", target_bir_lowering=False, debug=False,
                   num_devices=NCORES)

    # ---- I/O ----
    XT = nc.dram_tensor("xt", [D, S], f32, kind="ExternalInput").ap()
    XTQ = nc.dram_tensor("xtq", [D, RPC], f32, kind="ExternalInput").ap()
    KEYS = nc.dram_tensor("keys", [SS, D], f32, kind="ExternalInput").ap()
    VALS = nc.dram_tensor("vals", [SS, D], f32, kind="ExternalInput").ap()
    WQT = nc.dram_tensor("wqt", [D, D], f32, kind="ExternalInput").ap()
    WKT = nc.dram_tensor("wkt", [D, D], f32, kind="ExternalInput").ap()
    WVT = nc.dram_tensor("wvt", [D, D], f32, kind="ExternalInput").ap()
    WOT = nc.dram_tensor("wot", [D, D], f32, kind="ExternalInput").ap()
    WKKT = nc.dram_tensor("wkkt", [D, D], f32, kind="ExternalInput").ap()
    WPT = nc.dram_tensor("wpt", [D, D], f32, kind="ExternalInput").ap()
    WGT = nc.dram_tensor("wgt", [2 * D, 1], f32, kind="ExternalInput").ap()
    BG = nc.dram_tensor("bg", [1, 1], f32, kind="ExternalInput").ap()
    BPROJ = nc.dram_tensor("bproj", [D, 1], f32, kind="ExternalInput").ap()
    VALIDN = nc.dram_tensor("validn", [128, 3], f32, kind="ExternalInput").ap()
    OUT = nc.dram_tensor("out", [RPC, D], f32, kind="ExternalOutput").ap()

    # ---- DRAM scratch ----
    simsbuf = nc.dram_tensor("simsbuf", [RT, 2, 128, HNS], f32,
                             kind="Internal").ap()
    qn_b = nc.dram_tensor("qn_b", [2, D, RPC], bf16, kind="Internal").ap()
    qn_g = nc.dram_tensor("qn_g", [NCORES, 2, D, RPC], bf16, kind="Internal",
                          addr_space="Shared").ap()
    cand_b = nc.dram_tensor("cand_b", [RT, 128, 24], f32, kind="Internal").ap()
    cand_g = nc.dram_tensor("cand_g", [NCORES, RT, 128, 24], f32,
                            kind="Internal", addr_space="Shared").ap()
    rs_in = nc.dram_tensor("rs_in", [NCORES, 1152, RPC], f32,
                           kind="Internal").ap()
    rs_out = nc.dram_tensor("rs_out", [1152, RPC], f32, kind="Internal").ap()

    def rsqrt_newton(pool, n2, tagp):
        r0 = pool.tile([128, 1], f32, tag=tagp + "r0", name=tagp + "r0")
        nc.vector.reciprocal(r0[:], n2)
        y = pool.tile([128, 1], f32, tag=tagp + "y", name=tagp + "y")
        nc.scalar.activation(y[:], r0[:], AF.Sqrt)
        for it in ("a", "b"):
            yy = pool.tile([128, 1], f32, tag=tagp + it + "1",
                           name=tagp + it + "1")
            nc.vector.tensor_tensor(yy[:], y[:], y[:], op=AOT.mult)
            nc.vector.tensor_tensor(yy[:], n2, yy[:], op=AOT.mult)
            nc.vector.tensor_scalar(yy[:], yy[:], -0.5, scalar2=1.5,
                                    op0=AOT.mult, op1=AOT.add)
            y2 = pool.tile([128, 1], f32, tag=tagp + it + "2",
                           name=tagp + it + "2")
            nc.vector.tensor_tensor(y2[:], y[:], yy[:], op=AOT.mult)
            y = y2
        return y

    def split_hi_lo(src, hi, lo):
        nc.vector.tensor_copy(hi, src)
        nc.vector.tensor_tensor(lo, src, hi, op=AOT.subtract)

    with tile.TileContext(nc) as tc, ExitStack() as ctx:
        res = ctx.enter_context(tc.tile_pool(name="res", bufs=1))
        ident = res.tile([128, 128], bf16)
        make_identity(nc, ident)
        identf = res.tile([128, 128], f32)
        make_identity(nc, identf)
        maxes_all = res.tile([128, RT, 48], f32)
        t_m = res.tile([128, RT, 2], f32)
        attn_oT = res.tile([128, DT, RPC], f32)
        attn_oTb = res.tile([128, DT, RPC], bf16)
        dmask = res.tile([128, MT, 256], f32)

        # ================= P0: qn + AllGather + diag masks ================
        with tc.tile_pool(name="p0", bufs=2) as p0, \
             tc.tile_pool(name="ps0", bufs=2, space="PSUM") as ps0, \
             tc.tile_pool(name="ps0t", bufs=2, space="PSUM") as ps0t:
            # diagonal causal masks (additive 0/NEG), before any collective
            nc.vector.memset(dmask[:], 0.0)
            for m in range(MT):
                nc.gpsimd.affine_select(
                    dmask[:, m, :], dmask[:, m, :],
                    pattern=[[-1, 256]], compare_op=AOT.is_ge, fill=NEG,
                    base=m * 128, channel_multiplier=1)
            xq_hi = res.tile([128, DT, RPC], bf16)
            xq_lo = p0.tile([128, DT, RPC], bf16, tag="xql")
            wk_hi = p0.tile([128, DT, D], bf16, tag="wkh")
            wk_lo = p0.tile([128, DT, D], bf16, tag="wkl")
            for d in range(DT):
                t = p0.tile([128, RPC], f32, tag="ld")
                nc.sync.dma_start(t[:], XTQ[d * 128:(d + 1) * 128, :])
                split_hi_lo(t[:], xq_hi[:, d, :], xq_lo[:, d, :])
                tw = p0.tile([128, D], f32, tag="ldw")
                nc.sync.dma_start(tw[:], WKKT[d * 128:(d + 1) * 128, :])
                split_hi_lo(tw[:], wk_hi[:, d, :], wk_lo[:, d, :])
            for m in range(MT):
                qrow = p0.tile([128, D], f32, tag="qrow")
                for chn in range(2):
                    ps = ps0.tile([128, 512], f32, tag="mm")
                    cs = slice(chn * 512, (chn + 1) * 512)
                    ms = slice(m * 128, (m + 1) * 128)
                    for d in range(DT):
                        nc.tensor.matmul(ps[:], xq_hi[:, d, ms], wk_hi[:, d, cs],
                                         start=(d == 0), stop=False)
                        nc.tensor.matmul(ps[:], xq_hi[:, d, ms], wk_lo[:, d, cs],
                                         start=False, stop=False)
                        nc.tensor.matmul(ps[:], xq_lo[:, d, ms], wk_hi[:, d, cs],
                                         start=False, stop=(d == DT - 1))
                    nc.vector.tensor_copy(qrow[:, cs], ps[:])
                n2 = p0.tile([128, 1], f32, tag="n2")
                sq = p0.tile([128, D], f32, tag="sq")
                nc.scalar.activation(sq[:], qrow[:], AF.Square, accum_out=n2[:])
                inv = rsqrt_newton(p0, n2[:], "nq")
                nc.vector.tensor_scalar_mul(qrow[:], qrow[:], inv[:])
                q_hi = p0.tile([128, D], bf16, tag="qhi")
                q_lo = p0.tile([128, D], bf16, tag="qlo")
                split_hi_lo(qrow[:], q_hi[:], q_lo[:])
                for d in range(DT):
                    for hl, src in enumerate((q_hi, q_lo)):
                        tp = ps0t.tile([128, 128], bf16, tag="tp")
                        nc.tensor.transpose(tp[:], src[:, d * 128:(d + 1) * 128],
                                            ident[:])
                        sb = p0.tile([128, 128], bf16, tag="tpo")
                        nc.vector.tensor_copy(sb[:], tp[:])
                        nc.sync.dma_start(
                            qn_b[hl, d * 128:(d + 1) * 128,
                                 m * 128:(m + 1) * 128], sb[:])
            nc.gpsimd.collective_compute(
                "AllGather", AOT.bypass, replica_groups=RG,
                ins=[qn_b.opt()], outs=[qn_g.opt()])

        # ============ P1: keys + sims + local topk (both halves) ==========
        def keys_half(h, p1, p1b, p1c, ps1t, knT_hi, knT_lo):
            for i in range(HIT):
                kt = p1b.tile([128, D], f32, tag="kld", name="kld")
                nc.sync.dma_start(
                    kt[:], KEYS[h * HNS + i * 128:h * HNS + (i + 1) * 128, :])
                n2 = p1c.tile([128, 1], f32, tag="kn2", name="kn2")
                sq = p1c.tile([128, D], f32, tag="scr", name="ksq")
                nc.scalar.activation(sq[:], kt[:], AF.Square, accum_out=n2[:])
                inv = rsqrt_newton(p1c, n2[:], "nk")
                nc.vector.tensor_scalar_mul(kt[:], kt[:], inv[:])
                k_hi = p1b.tile([128, D], bf16, tag="khi", name="khi")
                k_lo = p1b.tile([128, D], bf16, tag="klo", name="klo")
                split_hi_lo(kt[:], k_hi[:], k_lo[:])
                for d in range(DT):
                    for src, dst in ((k_hi, knT_hi), (k_lo, knT_lo)):
                        tp = ps1t.tile([128, 128], bf16, tag="tp", name="tp")
                        nc.tensor.transpose(
                            tp[:], src[:, d * 128:(d + 1) * 128], ident[:])
                        if d % 2 == 0:
                            nc.scalar.activation(
                                dst[:, d, i * 128:(i + 1) * 128], tp[:], AF.Copy)
                        else:
                            nc.vector.tensor_copy(
                                dst[:, d, i * 128:(i + 1) * 128], tp[:])

        def sims_half(h, p1b, p1c, ps1, knT_hi, knT_lo, merge_local):
            for rt in range(RT):
                cb, ml = rt // 2, rt % 2
                qh = p1b.tile([128, DT, 128], bf16, tag="qsh", name="qsh")
                ql = p1b.tile([128, DT, 128], bf16, tag="qsl", name="qsl")
                for hl, dst in ((0, qh), (1, ql)):
                    nc.sync.dma_start(
                        dst[:],
                        qn_g[cb, hl, :, ml * 128:(ml + 1) * 128].rearrange(
                            "(dt p) c -> p dt c", p=128))
                sims = p1b.tile([128, HNS], f32, tag="sims", name="sims")
                pss = [ps1.tile([128, 512], f32, tag=f"mm{i}", name=f"mm{i}")
                       for i in range(4)]
                for d in range(DT):
                    for chn in range(4):
                        nc.tensor.matmul(pss[chn][:], qh[:, d, :],
                                         knT_hi[:, d, chn * 512:(chn + 1) * 512],
                                         start=(d == 0), stop=False)
                    for chn in range(4):
                        nc.tensor.matmul(pss[chn][:], qh[:, d, :],
                                         knT_lo[:, d, chn * 512:(chn + 1) * 512],
                                         start=False, stop=False)
                    for chn in range(4):
                        nc.tensor.matmul(pss[chn][:], ql[:, d, :],
                                         knT_hi[:, d, chn * 512:(chn + 1) * 512],
                                         start=False, stop=(d == DT - 1))
                for chn in range(4):
                    nc.scalar.activation(sims[:, chn * 512:(chn + 1) * 512],
                                         pss[chn][:], AF.Copy)
                nc.sync.dma_start(simsbuf[rt, h], sims[:])
                scr = p1c.tile([128, HNS], f32, tag="scr", name="scr")
                nc.vector.tensor_copy(scr[:], sims[:])
                for r in range(3):
                    mx = maxes_all[:, rt, h * 24 + r * 8:h * 24 + (r + 1) * 8]
                    nc.vector.max(mx, scr[:])
                    nc.vector.match_replace(scr[:], mx, scr[:], NEG)
                if merge_local:
                    mscr = p1c.tile([128, 48], f32, tag="mscr", name="mscr")
                    nc.vector.tensor_copy(mscr[:], maxes_all[:, rt, :])
                    loc = p1c.tile([128, 24], f32, tag="loc", name="loc")
                    for r in range(3):
                        nc.vector.max(loc[:, r * 8:(r + 1) * 8], mscr[:])
                        nc.vector.match_replace(mscr[:], loc[:, r * 8:(r + 1) * 8],
                                                mscr[:], NEG)
                    nc.sync.dma_start(cand_b[rt], loc[:])

        with tc.tile_pool(name="vp", bufs=2) as vpool:
            vals_bf0 = vpool.tile([128, HIT, D], bf16, tag="vals", name="vals0")
            for i in range(HIT):
                vt = vpool.tile([128, D], f32, tag="vld", name="vld")
                nc.sync.dma_start(vt[:], VALS[i * 128:(i + 1) * 128, :])
                nc.vector.tensor_copy(vals_bf0[:, i, :], vt[:])
            for h in range(2):
                with tc.tile_pool(name=f"p1_{h}", bufs=1) as p1, \
                     tc.tile_pool(name=f"p1b_{h}", bufs=2) as p1b, \
                     tc.tile_pool(name=f"p1c_{h}", bufs=1) as p1c, \
                     tc.tile_pool(name=f"ps1_{h}", bufs=1, space="PSUM") as ps1, \
                     tc.tile_pool(name=f"ps1t_{h}", bufs=2, space="PSUM") as ps1t:
                    knT_hi = p1.tile([128, DT, HNS], bf16, tag="knh", name="knh")
                    knT_lo = p1.tile([128, DT, HNS], bf16, tag="knl", name="knl")
                    keys_half(h, p1, p1b, p1c, ps1t, knT_hi, knT_lo)
                    sims_half(h, p1b, p1c, ps1, knT_hi, knT_lo, h == 1)
            nc.gpsimd.collective_compute(
                "AllGather", AOT.bypass, replica_groups=RG,
                ins=[cand_b.opt()], outs=[cand_g.opt()])

            # ============ P3: W, weighted matmul (merge inlined) ============
            with tc.tile_pool(name="p3", bufs=1) as p3, \
                 tc.tile_pool(name="p3b", bufs=2) as p3b, \
                 tc.tile_pool(name="ps3", bufs=4, space="PSUM") as ps3, \
                 tc.tile_pool(name="ps3t", bufs=2, space="PSUM") as ps3t:
                unorm = p3.tile([128, DT, ROWS], f32, tag="unorm")
                z_cols = p3.tile([128, RT], f32, tag="zc")
                for h in range(2):
                    if h == 0:
                        vals_bf = vals_bf0
                    else:
                        vals_bf = vpool.tile([128, HIT, D], bf16, tag="vals",
                                             name="vals1")
                        for i in range(HIT):
                            vt = vpool.tile([128, D], f32, tag="vld", name="vld")
                            nc.sync.dma_start(
                                vt[:],
                                VALS[HNS + i * 128:HNS + (i + 1) * 128, :])
                            nc.vector.tensor_copy(vals_bf[:, i, :], vt[:])
                    for g in range(4):
                        w_T = p3.tile([128, HIT, 512], bf16, tag="wT",
                                      name="wT")
                        for rl in range(4):
                            rt = g * 4 + rl
                            if h == 0:
                                gall = p3.tile([128, NCORES * 24], f32,
                                               tag="gall", name="gall")
                                for cb in range(NCORES):
                                    nc.sync.dma_start(
                                        gall[:, cb * 24:(cb + 1) * 24],
                                        cand_g[cb, rt])
                                gm = p3.tile([128, 4, 8], f32, tag="gm",
                                             name="gm")
                                for r in range(4):
                                    nc.vector.max(gm[:, r, :], gall[:])
                                    nc.vector.match_replace(gall[:], gm[:, r, :],
                                                            gall[:], NEG)
                                nc.vector.tensor_copy(t_m[:, rt, 0:1],
                                                      gm[:, 3, 7:8])
                                nc.vector.tensor_copy(t_m[:, rt, 1:2],
                                                      gm[:, 0, 0:1])
                            sims = p3.tile([128, HNS], f32, tag="srl",
                                           name="srl")
                            nc.sync.dma_start(sims[:], simsbuf[rt, h])
                            mbias = p3b.tile([128, 1], f32, tag="mb", name="mb")
                            nc.vector.tensor_scalar_mul(mbias[:],
                                                        t_m[:, rt, 1:2],
                                                        -1.0 / TEMP)
                            expw = p3b.tile([128, HNS], f32, tag="expw",
                                            name="expw")
                            nc.scalar.activation(expw[:], sims[:], AF.Exp,
                                                 bias=mbias[:], scale=1.0 / TEMP)
                            wmask = p3b.tile([128, HNS], bf16, tag="wm",
                                             name="wm")
                            z = p3b.tile([128, 1], f32, tag="z", name="z")
                            nc.vector.scalar_tensor_tensor(
                                wmask[:], sims[:], t_m[:, rt, 0:1], expw[:],
                                op0=AOT.is_ge, op1=AOT.mult, accum_out=z[:])
                            if h == 0:
                                nc.vector.tensor_copy(z_cols[:, rt:rt + 1], z[:])
                            else:
                                nc.vector.tensor_tensor(
                                    z_cols[:, rt:rt + 1], z_cols[:, rt:rt + 1],
                                    z[:], op=AOT.add)
                            for i in range(HIT):
                                tp = ps3t.tile([128, 128], bf16, tag="tp",
                                               name="tp")
                                nc.tensor.transpose(
                                    tp[:], wmask[:, i * 128:(i + 1) * 128],
                                    ident[:])
                                if i % 2 == 0:
                                    nc.scalar.activation(
                                        w_T[:, i, rl * 128:(rl + 1) * 128],
                                        tp[:], AF.Copy)
                                else:
                                    nc.vector.tensor_copy(
                                        w_T[:, i, rl * 128:(rl + 1) * 128],
                                        tp[:])
                        for d in range(DT):
                            ps = ps3.tile([128, 512], f32, tag="mm", name="mm")
                            for i in range(HIT):
                                nc.tensor.matmul(
                                    ps[:], vals_bf[:, i, d * 128:(d + 1) * 128],
                                    w_T[:, i, :], start=(i == 0),
                                    stop=(i == HIT - 1))
                            gs = slice(g * 512, (g + 1) * 512)
                            if h == 0:
                                nc.scalar.activation(unorm[:, d, gs], ps[:],
                                                     AF.Copy)
                            else:
                                nc.vector.tensor_tensor(unorm[:, d, gs],
                                                        unorm[:, d, gs], ps[:],
                                                        op=AOT.add)
                # pack rs_in
                for cb in range(NCORES):
                    for d in range(DT):
                        nc.sync.dma_start(
                            rs_in[cb, d * 128:(d + 1) * 128, :],
                            unorm[:, d, cb * RPC:(cb + 1) * RPC])
                zt = ps3t.tile([RT, 128], f32, tag="zt")
                nc.tensor.transpose(zt[:], z_cols[:], identf[:])
                zrow = p3.tile([RT, 128], f32, tag="zrow")
                nc.vector.tensor_copy(zrow[:], zt[:])
                zero = p3.tile([128, RPC], f32, tag="srl")
                nc.vector.memset(zero[:], 0.0)
                for cb in range(NCORES):
                    nc.sync.dma_start(
                        rs_in[cb, 1024:1025, :].rearrange(
                            "o (a b) -> (o a) b", a=2),
                        zrow[cb * 2:cb * 2 + 2, :])
                    nc.sync.dma_start(rs_in[cb, 1025:1152, :], zero[0:127, :])

        # ================= P4: causal attention (own block) ===============
        with tc.tile_pool(name="p4", bufs=1) as p4, \
             tc.tile_pool(name="p4b", bufs=2) as p4b, \
             tc.tile_pool(name="ps4", bufs=2, space="PSUM") as ps4, \
             tc.tile_pool(name="ps4s", bufs=2, space="PSUM") as ps4s, \
             tc.tile_pool(name="ps4p", bufs=2, space="PSUM") as ps4p, \
             tc.tile_pool(name="ps4t", bufs=2, space="PSUM") as ps4t:
            xb = p4.tile([128, DT, S], bf16, tag="xb")
            xqb = xq_hi
            wq = p4.tile([128, DT, D], bf16, tag="wq")
            wk = p4.tile([128, DT, D], bf16, tag="wk")
            wv = p4.tile([128, DT, D], bf16, tag="wv")
            wo = p4.tile([128, DT, D], bf16, tag="wo")
            vneg = p4.tile([128, 3], f32, tag="vneg")
            nc.sync.dma_start(vneg[:], VALIDN[:])
            nc.gpsimd.collective_compute(
                "ReduceScatter", AOT.add, replica_groups=RG,
                ins=[rs_in.opt()], outs=[rs_out.opt()])
            for d in range(DT):
                t = p4b.tile([128, S], f32, tag="xld")
                nc.sync.dma_start(t[:], XT[d * 128:(d + 1) * 128, :])
                nc.vector.tensor_copy(xb[:, d, :], t[:])
                for src, dst in ((WQT, wq), (WKT, wk), (WVT, wv), (WOT, wo)):
                    tw = p4b.tile([128, D], f32, tag="wld")
                    nc.sync.dma_start(tw[:], src[d * 128:(d + 1) * 128, :])
                    nc.vector.tensor_copy(dst[:, d, :], tw[:])
            qT = p4.tile([128, DT, RPC], bf16, tag="qT")
            kTp = p4.tile([128, DT, 768], bf16, tag="kTp")
            kTd = p4.tile([128, DT, RPC], bf16, tag="kTd")
            for d in range(DT):
                ps = ps4.tile([128, 512], f32, tag="pj")
                nc_d = slice(d * 128, (d + 1) * 128)
                for e in range(DT):
                    nc.tensor.matmul(ps[:, :RPC], wq[:, e, nc_d], xqb[:, e, :],
                                     start=(e == 0), stop=(e == DT - 1))
                nc.vector.tensor_copy(qT[:, d, :], ps[:, :RPC])
                for e in range(DT):
                    nc.tensor.matmul(ps[:, :RPC], wk[:, e, nc_d], xqb[:, e, :],
                                     start=(e == 0), stop=(e == DT - 1))
                nc.vector.tensor_copy(kTd[:, d, :], ps[:, :RPC])
                for chn in range(2):
                    cs = slice(chn * 512, min((chn + 1) * 512, 768))
                    w_ = cs.stop - cs.start
                    for e in range(DT):
                        nc.tensor.matmul(ps[:, :w_], wk[:, e, nc_d],
                                         xb[:, e, cs], start=(e == 0),
                                         stop=(e == DT - 1))
                    nc.vector.tensor_copy(kTp[:, d, cs], ps[:, :w_])
            vR = p4.tile([128, 6, D], bf16, tag="vR")
            vRd = p4.tile([128, MT, D], bf16, tag="vRd")
            for k in range(6):
                for chn in range(2):
                    ps = ps4.tile([128, 512], f32, tag="pj")
                    cs = slice(chn * 512, (chn + 1) * 512)
                    for e in range(DT):
                        nc.tensor.matmul(ps[:], xb[:, e, k * 128:(k + 1) * 128],
                                         wv[:, e, cs], start=(e == 0),
                                         stop=(e == DT - 1))
                    nc.vector.tensor_copy(vR[:, k, cs], ps[:])
            for m in range(MT):
                for chn in range(2):
                    ps = ps4.tile([128, 512], f32, tag="pj")
                    cs = slice(chn * 512, (chn + 1) * 512)
                    for e in range(DT):
                        nc.tensor.matmul(ps[:], xqb[:, e, m * 128:(m + 1) * 128],
                                         wv[:, e, cs], start=(e == 0),
                                         stop=(e == DT - 1))
                    nc.vector.tensor_copy(vRd[:, m, cs], ps[:])
            acT = p4.tile([128, DT, RPC], bf16, tag="acT")
            for hh in range(H):
                d, half = hh // 2, hh % 2
                hp = slice(64 * half, 64 * half + 64)
                for m in range(MT):
                    sc = p4b.tile([128, 1024], f32, tag="sc")
                    for chn in range(3):
                        ps = ps4s.tile([128, 256], f32, tag="sps")
                        nc.tensor.matmul(ps[:], qT[hp, d, m * 128:(m + 1) * 128],
                                         kTp[hp, d, chn * 256:(chn + 1) * 256])
                        nc.vector.tensor_scalar(
                            sc[:, chn * 256:(chn + 1) * 256], ps[:],
                            vneg[:, chn:chn + 1], scalar2=None, op0=AOT.add)
                    ps = ps4s.tile([128, 256], f32, tag="sps")
                    nc.tensor.matmul(ps[:], qT[hp, d, m * 128:(m + 1) * 128],
                                     kTd[hp, d, :])
                    nc.vector.tensor_tensor(sc[:, 768:1024], ps[:],
                                            dmask[:, m, :], op=AOT.add)
                    rm = p4b.tile([128, 1], f32, tag="rm")
                    nc.vector.tensor_reduce(rm[:], sc[:],
                                            axis=mybir.AxisListType.XYZW,
                                            op=AOT.max)
                    nc.vector.tensor_scalar_mul(rm[:], rm[:], -0.125)
                    ex = p4b.tile([128, 1024], f32, tag="ex")
                    zr = p4b.tile([128, 1], f32, tag="zr")
                    nc.scalar.activation(ex[:], sc[:], AF.Exp, bias=rm[:],
                                         scale=0.125, accum_out=zr[:])
                    iz = p4b.tile([128, 1], f32, tag="iz")
                    nc.vector.reciprocal(iz[:], zr[:])
                    ab = p4b.tile([128, 1024], bf16, tag="ab")
                    nc.scalar.activation(ab[:], ex[:], AF.Copy, scale=iz[:])
                    pv = ps4p.tile([128, 128], f32, tag="pv")
                    for k in range(8):
                        tp = ps4t.tile([128, 128], bf16, tag="tp")
                        nc.tensor.transpose(tp[:], ab[:, k * 128:(k + 1) * 128],
                                            ident[:])
                        at = p4b.tile([128, 128], bf16, tag="at")
                        nc.vector.tensor_copy(at[:], tp[:])
                        vsrc = (vR[:, k, hh * 64:(hh + 1) * 64] if k < 6 else
                                vRd[:, k - 6, hh * 64:(hh + 1) * 64])
                        nc.tensor.matmul(pv[hp, :], vsrc, at[:],
                                         start=(k == 0), stop=(k == 7))
                    nc.vector.tensor_copy(acT[hp, d, m * 128:(m + 1) * 128],
                                          pv[hp, :])
            for d in range(DT):
                ps = ps4.tile([128, 512], f32, tag="pj")
                for e in range(DT):
                    nc.tensor.matmul(ps[:, :RPC], wo[:, e, d * 128:(d + 1) * 128],
                                     acT[:, e, :], start=(e == 0),
                                     stop=(e == DT - 1))
                nc.vector.tensor_copy(attn_oT[:, d, :], ps[:, :RPC])
                nc.vector.tensor_copy(attn_oTb[:, d, :], ps[:, :RPC])

        # ================= P5: finale =================
        with tc.tile_pool(name="p5", bufs=1) as p5, \
             tc.tile_pool(name="p5b", bufs=2) as p5b, \
             tc.tile_pool(name="ps5", bufs=2, space="PSUM") as ps5, \
             tc.tile_pool(name="ps5t", bufs=2, space="PSUM") as ps5t:
            knn_s = p5.tile([128, DT, RPC], f32, tag="kns")
            for d in range(DT):
                nc.sync.dma_start(knn_s[:, d, :],
                                  rs_out[d * 128:(d + 1) * 128, :])
            zg = p5.tile([1, RPC], f32, tag="zg")
            nc.sync.dma_start(zg[:], rs_out[1024:1025, :])
            iz = p5.tile([1, RPC], f32, tag="izg")
            nc.vector.reciprocal(iz[:], zg[:])
            ones_r = p5.tile([1, 128], f32, tag="ones")
            nc.vector.memset(ones_r[:], 1.0)
            bc = ps5.tile([128, RPC], f32, tag="bc")
            nc.tensor.matmul(bc[:], ones_r[:], iz[:])
            izb = p5.tile([128, RPC], f32, tag="izb")
            nc.vector.tensor_copy(izb[:], bc[:])
            knn_nb = p5.tile([128, DT, RPC], bf16, tag="knnb")
            for d in range(DT):
                nc.vector.tensor_tensor(knn_s[:, d, :], knn_s[:, d, :], izb[:],
                                        op=AOT.mult)
                nc.vector.tensor_copy(knn_nb[:, d, :], knn_s[:, d, :])
            wp = p5.tile([128, DT, D], bf16, tag="wp")
            bpr = p5.tile([128, DT], f32, tag="bpr")
            for d in range(DT):
                tw = p5b.tile([128, D], f32, tag="wpld")
                nc.sync.dma_start(tw[:], WPT[d * 128:(d + 1) * 128, :])
                nc.vector.tensor_copy(wp[:, d, :], tw[:])
                nc.sync.dma_start(bpr[:, d:d + 1],
                                  BPROJ[d * 128:(d + 1) * 128, :])
            knn_oT = p5.tile([128, DT, RPC], f32, tag="knoT")
            knn_oTb = p5.tile([128, DT, RPC], bf16, tag="knoTb")
            for d in range(DT):
                ps = ps5.tile([128, RPC], f32, tag="pmm")
                for e in range(DT):
                    nc.tensor.matmul(ps[:], wp[:, e, d * 128:(d + 1) * 128],
                                     knn_nb[:, e, :], start=(e == 0),
                                     stop=(e == DT - 1))
                nc.vector.tensor_scalar(knn_oT[:, d, :], ps[:],
                                        bpr[:, d:d + 1], scalar2=None,
                                        op0=AOT.add)
                nc.vector.tensor_copy(knn_oTb[:, d, :], knn_oT[:, d, :])
            wg = p5.tile([128, 2 * DT], bf16, tag="wg")
            wgf = p5.tile([128, 2 * DT], f32, tag="wgf")
            nc.sync.dma_start(wgf[:],
                              WGT[:].rearrange("(a p) 1 -> p a", p=128))
            nc.vector.tensor_copy(wg[:], wgf[:])
            bgt = p5.tile([1, 1], f32, tag="bgt")
            nc.sync.dma_start(bgt[:], BG[:])
            gps = ps5.tile([1, RPC], f32, tag="gps")
            for e in range(DT):
                nc.tensor.matmul(gps[:], wg[:, e:e + 1], attn_oTb[:, e, :],
                                 start=(e == 0), stop=False)
            for e in range(DT):
                nc.tensor.matmul(gps[:], wg[:, DT + e:DT + e + 1],
                                 knn_oTb[:, e, :], start=False,
                                 stop=(e == DT - 1))
            gate = p5.tile([1, RPC], f32, tag="gate")
            nc.scalar.activation(gate[:], gps[:], AF.Sigmoid, bias=bgt[0:1, 0:1])
            gbc = ps5.tile([128, RPC], f32, tag="bc")
            nc.tensor.matmul(gbc[:], ones_r[:], gate[:])
            gateb = p5.tile([128, RPC], f32, tag="gateb")
            nc.vector.tensor_copy(gateb[:], gbc[:])
            for d in range(DT):
                dif = p5b.tile([128, RPC], f32, tag="dif")
                nc.vector.tensor_tensor(dif[:], attn_oT[:, d, :],
                                        knn_oT[:, d, :], op=AOT.subtract)
                nc.vector.tensor_tensor(dif[:], dif[:], gateb[:], op=AOT.mult)
                nc.vector.tensor_tensor(knn_oT[:, d, :], knn_oT[:, d, :],
                                        dif[:], op=AOT.add)
            for m in range(MT):
                orow = p5b.tile([128, D], f32, tag="orow")
                for d in range(DT):
                    tp = ps5t.tile([128, 128], f32, tag="tp")
                    nc.tensor.transpose(tp[:],
                                        knn_oT[:, d, m * 128:(m + 1) * 128],
                                        identf[:])
                    nc.vector.tensor_copy(orow[:, d * 128:(d + 1) * 128], tp[:])
                nc.sync.dma_start(OUT[m * 128:(m + 1) * 128, :], orow[:])

    nc.compile()
    _BUILT["nc"] = nc
    return nc


def kernel(x, store_keys, store_vals, Wq, Wk, Wv, Wo, Wkk, Wproj, bproj,
           Wg, bg):
    import os
    x = np.asarray(x, np.float32)
    store_keys = np.asarray(store_keys, np.float32)
    store_vals = np.asarray(store_vals, np.float32)
    Wq, Wk, Wv, Wo, Wkk, Wproj = (np.asarray(w, np.float32)
                                  for w in (Wq, Wk, Wv, Wo, Wkk, Wproj))
    bproj = np.asarray(bproj, np.float32)
    Wg = np.asarray(Wg, np.float32)
    bg = np.asarray(bg, np.float32)

    nc = _build()

    xtb = [np.ascontiguousarray(x[b].T) for b in range(B)]
    wqt = np.ascontiguousarray(Wq.T)
    wkt = np.ascontiguousarray(Wk.T)
    wvt = np.ascontiguousarray(Wv.T)
    wot = np.ascontiguousarray(Wo.T)
    wkkt = np.ascontiguousarray(Wkk.T)
    wpt = np.ascontiguousarray(Wproj.T)
    wgt = np.ascontiguousarray(Wg[0].reshape(2 * D, 1))
    bgt = bg.reshape(1, 1)
    bprojc = np.ascontiguousarray(bproj.reshape(D, 1))

    in_maps = []
    for c in range(NCORES):
        b, blk = c // 4, c % 4
        q0 = blk * 256
        validn = np.zeros((128, 3), np.float32)
        for ch in range(3):
            if (ch + 1) * 256 > q0:
                validn[:, ch] = NEG
        in_maps.append({
            "xt": xtb[b],
            "xtq": np.ascontiguousarray(xtb[b][:, q0:q0 + RPC]),
            "keys": store_keys[c * SS:(c + 1) * SS],
            "vals": store_vals[c * SS:(c + 1) * SS],
            "wqt": wqt, "wkt": wkt, "wvt": wvt, "wot": wot,
            "wkkt": wkkt, "wpt": wpt, "wgt": wgt, "bg": bgt,
            "bproj": bprojc, "validn": validn,
        })

    from concourse.bass_utils import run_bass_kernel_spmd
    trace = bool(os.environ.get("KNN_TRACE"))
    res = run_bass_kernel_spmd(nc, in_maps, list(range(NCORES)), trace=trace,
                               tmpdir=os.environ.get("KNN_TRACE_DIR"))
    if trace:
        _BUILT["exec_time_ns"] = res.exec_time_ns
    out = np.concatenate([res.results[c]["out"] for c in range(NCORES)], axis=0)
    return out.reshape(B, S, D).astype(np.float32)


# revision 13
# speedup vs baseline: 1.1084x; 1.0311x over previous
"""KNN-attention layer on 8 Trainium2 NeuronCores (Bass/Tile).

Sharding: core c owns query rows [256c, 256c+256) (batch c//4) and store
shard [4096c, 4096(c+1)).

Per-core program (static, identical on all cores):
  P0  knn-query projection (bf16 hi/lo split-3 matmul for fp32-exact sims),
      normalize, transpose, AllGather qn across cores; build diag causal masks.
  P1  per store half (2048): normalize+split+transpose keys; sims = qn @ kn.T
      for all 2048 rows via split-3; spill sims to DRAM; local top-32 per half
      (max8 + match_replace); in half 1, merge halves and AllGather candidates.
  P2  global threshold t (32nd) and max m per row from gathered candidates.
  P3  per half: reload sims, W = (s >= t) * exp((s-m)/T) in bf16 (+Z accum),
      transpose W, weighted matmul with store_vals -> unnormalized knn partial
      [1024, 2048]; pack + ReduceScatter (sum over cores, scatter by row block).
  P4  causal self-attention for own row block (bf16), static chunking:
      3 absolute 256-wide key pre-chunks gated by a per-core validity column
      + 1 diagonal chunk from the own-block x slice. Overlaps the RS (no
      gpsimd ops here; collectives own the gpsimd queue).
  P5  normalize knn by Z, project (+bias), gated combine, transpose to
      row-major, output [256, 1024] per core.

Selection precision: sims are computed as q_hi@k_hi + q_hi@k_lo + q_lo@k_hi
(bf16 splits, fp32 PSUM accumulation) which matches fp32 sims to ~1e-7 —
required because the global top-32 set must match the fp32 reference's.
"""
import sys

sys.path.insert(0, "/opt/trn_rl_repo")

import numpy as np

# ---------------- problem constants ----------------
B, S, D = 2, 1024, 1024
H, HD = 16, 64
N = 32768
KNN_K = 32
TEMP = 0.1
NCORES = 8
ROWS = B * S                # 2048
RPC = ROWS // NCORES        # 256 rows per core
SS = N // NCORES            # 4096 stores per core
HNS = SS // 2               # 2048 per half
DT = D // 128               # 8 feature tiles
RT = ROWS // 128            # 16 global row tiles
MT = RPC // 128             # 2 own row tiles
HIT = HNS // 128            # 16 store tiles per half
NEG = -1e30
RG = [list(range(NCORES))]

_BUILT = {}


def _build():
    if "nc" in _BUILT:
        return _BUILT["nc"]
    from contextlib import ExitStack
    import concourse.bass as bass
    import concourse.tile as tile
    from concourse import bacc, mybir
    from concourse.masks import make_identity

    f32 = mybir.dt.float32
    bf16 = mybir.dt.bfloat16
    AOT = mybir.AluOpType
    AF = mybir.ActivationFunctionType

    nc = bacc.Bacc("TRN2", target_bir_lowering=False, debug=False,
                   num_devices=NCORES)

    # ---- I/O ----
    XT = nc.dram_tensor("xt", [D, S], f32, kind="ExternalInput").ap()
    XTQ = nc.dram_tensor("xtq", [D, RPC], f32, kind="ExternalInput").ap()
    KEYS = nc.dram_tensor("keys", [SS, D], f32, kind="ExternalInput").ap()
    VALS = nc.dram_tensor("vals", [SS, D], f32, kind="ExternalInput").ap()
    WQT = nc.dram_tensor("wqt", [D, D], f32, kind="ExternalInput").ap()
    WKT = nc.dram_tensor("wkt", [D, D], f32, kind="ExternalInput").ap()
    WVT = nc.dram_tensor("wvt", [D, D], f32, kind="ExternalInput").ap()
    WOT = nc.dram_tensor("wot", [D, D], f32, kind="ExternalInput").ap()
    WKKT = nc.dram_tensor("wkkt", [D, D], f32, kind="ExternalInput").ap()
    WPT = nc.dram_tensor("wpt", [D, D], f32, kind="ExternalInput").ap()
    WGT = nc.dram_tensor("wgt", [2 * D, 1], f32, kind="ExternalInput").ap()
    BG = nc.dram_tensor("bg", [1, 1], f32, kind="ExternalInput").ap()
    BPROJ = nc.dram_tensor("bproj", [D, 1], f32, kind="ExternalInput").ap()
    VALIDN = nc.dram_tensor("validn", [128, 3], f32, kind="ExternalInput").ap()
    OUT = nc.dram_tensor("out", [RPC, D], f32, kind="ExternalOutput").ap()

    # ---- DRAM scratch ----
    simsbuf = nc.dram_tensor("simsbuf", [RT, 2, 128, HNS], f32,
                             kind="Internal").ap()
    qn_b = nc.dram_tensor("qn_b", [2, D, RPC], bf16, kind="Internal").ap()
    qn_g = nc.dram_tensor("qn_g", [NCORES, 2, D, RPC], bf16, kind="Internal",
                          addr_space="Shared").ap()
    cand_b = nc.dram_tensor("cand_b", [RT, 128, 24], f32, kind="Internal").ap()
    cand_g = nc.dram_tensor("cand_g", [NCORES, RT, 128, 24], f32,
                            kind="Internal", addr_space="Shared").ap()
    rs_in = nc.dram_tensor("rs_in", [NCORES, 1152, RPC], f32,
                           kind="Internal").ap()
    rs_out = nc.dram_tensor("rs_out", [1152, RPC], f32, kind="Internal").ap()

    def rsqrt_newton(pool, n2, tagp):
        r0 = pool.tile([128, 1], f32, tag=tagp + "r0", name=tagp + "r0")
        nc.vector.reciprocal(r0[:], n2)
        y = pool.tile([128, 1], f32, tag=tagp + "y", name=tagp + "y")
        nc.scalar.activation(y[:], r0[:], AF.Sqrt)
        for it in ("a", "b"):
            yy = pool.tile([128, 1], f32, tag=tagp + it + "1",
                           name=tagp + it + "1")
            nc.vector.tensor_tensor(yy[:], y[:], y[:], op=AOT.mult)
            nc.vector.tensor_tensor(yy[:], n2, yy[:], op=AOT.mult)
            nc.vector.tensor_scalar(yy[:], yy[:], -0.5, scalar2=1.5,
                                    op0=AOT.mult, op1=AOT.add)
            y2 = pool.tile([128, 1], f32, tag=tagp + it + "2",
                           name=tagp + it + "2")
            nc.vector.tensor_tensor(y2[:], y[:], yy[:], op=AOT.mult)
            y = y2
        return y

    def split_hi_lo(src, hi, lo):
        nc.vector.tensor_copy(hi, src)
        nc.vector.tensor_tensor(lo, src, hi, op=AOT.subtract)

    with tile.TileContext(nc) as tc, ExitStack() as ctx:
        res = ctx.enter_context(tc.tile_pool(name="res", bufs=1))
        ident = res.tile([128, 128], bf16)
        make_identity(nc, ident)
        identf = res.tile([128, 128], f32)
        make_identity(nc, identf)
        maxes_all = res.tile([128, RT, 48], f32)
        t_m = res.tile([128, RT, 2], f32)
        attn_oT = res.tile([128, DT, RPC], f32)
        attn_oTb = res.tile([128, DT, RPC], bf16)
        dmask = res.tile([128, MT, 256], f32)

        # ================= P0: qn + AllGather + diag masks ================
        with tc.tile_pool(name="p0", bufs=2) as p0, \
             tc.tile_pool(name="ps0", bufs=2, space="PSUM") as ps0, \
             tc.tile_pool(name="ps0t", bufs=2, space="PSUM") as ps0t:
            # diagonal causal masks (additive 0/NEG), before any collective
            nc.vector.memset(dmask[:], 0.0)
            for m in range(MT):
                nc.gpsimd.affine_select(
                    dmask[:, m, :], dmask[:, m, :],
                    pattern=[[-1, 256]], compare_op=AOT.is_ge, fill=NEG,
                    base=m * 128, channel_multiplier=1)
            xq_hi = res.tile([128, DT, RPC], bf16)
            xq_lo = p0.tile([128, DT, RPC], bf16, tag="xql")
            wk_hi = p0.tile([128, DT, D], bf16, tag="wkh")
            wk_lo = p0.tile([128, DT, D], bf16, tag="wkl")
            for d in range(DT):
                t = p0.tile([128, RPC], f32, tag="ld")
                nc.sync.dma_start(t[:], XTQ[d * 128:(d + 1) * 128, :])
                split_hi_lo(t[:], xq_hi[:, d, :], xq_lo[:, d, :])
                tw = p0.tile([128, D], f32, tag="ldw")
                nc.sync.dma_start(tw[:], WKKT[d * 128:(d + 1) * 128, :])
                split_hi_lo(tw[:], wk_hi[:, d, :], wk_lo[:, d, :])
            for m in range(MT):
                qrow = p0.tile([128, D], f32, tag="qrow")
                for chn in range(2):
                    ps = ps0.tile([128, 512], f32, tag="mm")
                    cs = slice(chn * 512, (chn + 1) * 512)
                    ms = slice(m * 128, (m + 1) * 128)
                    for d in range(DT):
                        nc.tensor.matmul(ps[:], xq_hi[:, d, ms], wk_hi[:, d, cs],
                                         start=(d == 0), stop=False)
                        nc.tensor.matmul(ps[:], xq_hi[:, d, ms], wk_lo[:, d, cs],
                                         start=False, stop=False)
                        nc.tensor.matmul(ps[:], xq_lo[:, d, ms], wk_hi[:, d, cs],
                                         start=False, stop=(d == DT - 1))
                    nc.vector.tensor_copy(qrow[:, cs], ps[:])
                n2 = p0.tile([128, 1], f32, tag="n2")
                sq = p0.tile([128, D], f32, tag="sq")
                nc.scalar.activation(sq[:], qrow[:], AF.Square, accum_out=n2[:])
                inv = rsqrt_newton(p0, n2[:], "nq")
                nc.vector.tensor_scalar_mul(qrow[:], qrow[:], inv[:])
                q_hi = p0.tile([128, D], bf16, tag="qhi")
                q_lo = p0.tile([128, D], bf16, tag="qlo")
                split_hi_lo(qrow[:], q_hi[:], q_lo[:])
                for d in range(DT):
                    for hl, src in enumerate((q_hi, q_lo)):
                        tp = ps0t.tile([128, 128], bf16, tag="tp")
                        nc.tensor.transpose(tp[:], src[:, d * 128:(d + 1) * 128],
                                            ident[:])
                        sb = p0.tile([128, 128], bf16, tag="tpo")
                        nc.vector.tensor_copy(sb[:], tp[:])
                        nc.sync.dma_start(
                            qn_b[hl, d * 128:(d + 1) * 128,
                                 m * 128:(m + 1) * 128], sb[:])
            nc.gpsimd.collective_compute(
                "AllGather", AOT.bypass, replica_groups=RG,
                ins=[qn_b.opt()], outs=[qn_g.opt()])

        # ============ P1: keys + sims + local topk (both halves) ==========
        def keys_half(h, p1, p1b, p1c, ps1t, knT_hi, knT_lo):
            for i in range(HIT):
                kt = p1b.tile([128, D], f32, tag="kld", name="kld")
                nc.sync.dma_start(
                    kt[:], KEYS[h * HNS + i * 128:h * HNS + (i + 1) * 128, :])
                n2 = p1c.tile([128, 1], f32, tag="kn2", name="kn2")
                sq = p1c.tile([128, D], f32, tag="scr", name="ksq")
                nc.scalar.activation(sq[:], kt[:], AF.Square, accum_out=n2[:])
                inv = rsqrt_newton(p1c, n2[:], "nk")
                nc.vector.tensor_scalar_mul(kt[:], kt[:], inv[:])
                k_hi = p1b.tile([128, D], bf16, tag="khi", name="khi")
                k_lo = p1b.tile([128, D], bf16, tag="klo", name="klo")
                split_hi_lo(kt[:], k_hi[:], k_lo[:])
                for d in range(DT):
                    for src, dst in ((k_hi, knT_hi), (k_lo, knT_lo)):
                        tp = ps1t.tile([128, 128], bf16, tag="tp", name="tp")
                        nc.tensor.transpose(
                            tp[:], src[:, d * 128:(d + 1) * 128], ident[:])
                        if d % 2 == 0:
                            nc.scalar.activation(
                                dst[:, d, i * 128:(i + 1) * 128], tp[:], AF.Copy)
                        else:
                            nc.vector.tensor_copy(
                                dst[:, d, i * 128:(i + 1) * 128], tp[:])

        def sims_half(h, p1b, p1c, ps1, knT_hi, knT_lo, merge_local):
            for rt in range(RT):
                cb, ml = rt // 2, rt % 2
                qh = p1b.tile([128, DT, 128], bf16, tag="qsh", name="qsh")
                ql = p1b.tile([128, DT, 128], bf16, tag="qsl", name="qsl")
                for hl, dst in ((0, qh), (1, ql)):
                    nc.sync.dma_start(
                        dst[:],
                        qn_g[cb, hl, :, ml * 128:(ml + 1) * 128].rearrange(
                            "(dt p) c -> p dt c", p=128))
                sims = p1b.tile([128, HNS], f32, tag="sims", name="sims")
                pss = [ps1.tile([128, 512], f32, tag=f"mm{i}", name=f"mm{i}")
                       for i in range(4)]
                for d in range(DT):
                    for chn in range(4):
                        nc.tensor.matmul(pss[chn][:], qh[:, d, :],
                                         knT_hi[:, d, chn * 512:(chn + 1) * 512],
                                         start=(d == 0), stop=False)
                    for chn in range(4):
                        nc.tensor.matmul(pss[chn][:], qh[:, d, :],
                                         knT_lo[:, d, chn * 512:(chn + 1) * 512],
                                         start=False, stop=False)
                    for chn in range(4):
                        nc.tensor.matmul(pss[chn][:], ql[:, d, :],
                                         knT_hi[:, d, chn * 512:(chn + 1) * 512],
                                         start=False, stop=(d == DT - 1))
                for chn in range(4):
                    nc.scalar.activation(sims[:, chn * 512:(chn + 1) * 512],
                                         pss[chn][:], AF.Copy)
                nc.sync.dma_start(simsbuf[rt, h], sims[:])
                scr = p1c.tile([128, HNS], f32, tag="scr", name="scr")
                nc.vector.tensor_copy(scr[:], sims[:])
                for r in range(3):
                    mx = maxes_all[:, rt, h * 24 + r * 8:h * 24 + (r + 1) * 8]
                    nc.vector.max(mx, scr[:])
                    nc.vector.match_replace(scr[:], mx, scr[:], NEG)
                if merge_local:
                    mscr = p1c.tile([128, 48], f32, tag="mscr", name="mscr")
                    nc.vector.tensor_copy(mscr[:], maxes_all[:, rt, :])
                    loc = p1c.tile([128, 24], f32, tag="loc", name="loc")
                    for r in range(3):
                        nc.vector.max(loc[:, r * 8:(r + 1) * 8], mscr[:])
                        nc.vector.match_replace(mscr[:], loc[:, r * 8:(r + 1) * 8],
                                                mscr[:], NEG)
                    nc.sync.dma_start(cand_b[rt], loc[:])

        with tc.tile_pool(name="vp", bufs=2) as vpool:
            vals_bf0 = vpool.tile([128, HIT, D], bf16, tag="vals", name="vals0")
            for i in range(HIT):
                vt = vpool.tile([128, D], f32, tag="vld", name="vld")
                nc.sync.dma_start(vt[:], VALS[i * 128:(i + 1) * 128, :])
                nc.vector.tensor_copy(vals_bf0[:, i, :], vt[:])
            for h in range(2):
                with tc.tile_pool(name=f"p1_{h}", bufs=1) as p1, \
                     tc.tile_pool(name=f"p1b_{h}", bufs=2) as p1b, \
                     tc.tile_pool(name=f"p1c_{h}", bufs=1) as p1c, \
                     tc.tile_pool(name=f"ps1_{h}", bufs=1, space="PSUM") as ps1, \
                     tc.tile_pool(name=f"ps1t_{h}", bufs=2, space="PSUM") as ps1t:
                    knT_hi = p1.tile([128, DT, HNS], bf16, tag="knh", name="knh")
                    knT_lo = p1.tile([128, DT, HNS], bf16, tag="knl", name="knl")
                    keys_half(h, p1, p1b, p1c, ps1t, knT_hi, knT_lo)
                    sims_half(h, p1b, p1c, ps1, knT_hi, knT_lo, h == 1)
            nc.gpsimd.collective_compute(
                "AllGather", AOT.bypass, replica_groups=RG,
                ins=[cand_b.opt()], outs=[cand_g.opt()])

            # ============ P3: W, weighted matmul (merge inlined) ============
            with tc.tile_pool(name="p3", bufs=1) as p3, \
                 tc.tile_pool(name="p3b", bufs=2) as p3b, \
                 tc.tile_pool(name="ps3", bufs=4, space="PSUM") as ps3, \
                 tc.tile_pool(name="ps3t", bufs=2, space="PSUM") as ps3t:
                unorm = p3.tile([128, DT, ROWS], f32, tag="unorm")
                z_cols = p3.tile([128, RT], f32, tag="zc")
                for h in range(2):
                    if h == 0:
                        vals_bf = vals_bf0
                    else:
                        vals_bf = vpool.tile([128, HIT, D], bf16, tag="vals",
                                             name="vals1")
                        for i in range(HIT):
                            vt = vpool.tile([128, D], f32, tag="vld", name="vld")
                            nc.sync.dma_start(
                                vt[:],
                                VALS[HNS + i * 128:HNS + (i + 1) * 128, :])
                            nc.vector.tensor_copy(vals_bf[:, i, :], vt[:])
                    for g in range(4):
                        w_T = p3.tile([128, HIT, 512], bf16, tag="wT",
                                      name="wT")
                        for rl in range(4):
                            rt = g * 4 + rl
                            if h == 0:
                                gall = p3.tile([128, NCORES * 24], f32,
                                               tag="gall", name="gall")
                                for cb in range(NCORES):
                                    nc.sync.dma_start(
                                        gall[:, cb * 24:(cb + 1) * 24],
                                        cand_g[cb, rt])
                                gm = p3.tile([128, 4, 8], f32, tag="gm",
                                             name="gm")
                                for r in range(4):
                                    nc.vector.max(gm[:, r, :], gall[:])
                                    nc.vector.match_replace(gall[:], gm[:, r, :],
                                                            gall[:], NEG)
                                nc.vector.tensor_copy(t_m[:, rt, 0:1],
                                                      gm[:, 3, 7:8])
                                nc.vector.tensor_copy(t_m[:, rt, 1:2],
                                                      gm[:, 0, 0:1])
                            sims = p3.tile([128, HNS], f32, tag="srl",
                                           name="srl")
                            nc.sync.dma_start(sims[:], simsbuf[rt, h])
                            mbias = p3b.tile([128, 1], f32, tag="mb", name="mb")
                            nc.vector.tensor_scalar_mul(mbias[:],
                                                        t_m[:, rt, 1:2],
                                                        -1.0 / TEMP)
                            expw = p3b.tile([128, HNS], f32, tag="expw",
                                            name="expw")
                            nc.scalar.activation(expw[:], sims[:], AF.Exp,
                                                 bias=mbias[:], scale=1.0 / TEMP)
                            wmask = p3b.tile([128, HNS], bf16, tag="wm",
                                             name="wm")
                            z = p3b.tile([128, 1], f32, tag="z", name="z")
                            nc.vector.scalar_tensor_tensor(
                                wmask[:], sims[:], t_m[:, rt, 0:1], expw[:],
                                op0=AOT.is_ge, op1=AOT.mult, accum_out=z[:])
                            if h == 0:
                                nc.vector.tensor_copy(z_cols[:, rt:rt + 1], z[:])
                            else:
                                nc.vector.tensor_tensor(
                                    z_cols[:, rt:rt + 1], z_cols[:, rt:rt + 1],
                                    z[:], op=AOT.add)
                            for i in range(HIT):
                                tp = ps3t.tile([128, 128], bf16, tag="tp",
                                               name="tp")
                                nc.tensor.transpose(
                                    tp[:], wmask[:, i * 128:(i + 1) * 128],
                                    ident[:])
                                if i % 2 == 0:
                                    nc.scalar.activation(
                                        w_T[:, i, rl * 128:(rl + 1) * 128],
                                        tp[:], AF.Copy)
                                else:
                                    nc.vector.tensor_copy(
                                        w_T[:, i, rl * 128:(rl + 1) * 128],
                                        tp[:])
                        for d in range(DT):
                            ps = ps3.tile([128, 512], f32, tag="mm", name="mm")
                            for i in range(HIT):
                                nc.tensor.matmul(
                                    ps[:], vals_bf[:, i, d * 128:(d + 1) * 128],
                                    w_T[:, i, :], start=(i == 0),
                                    stop=(i == HIT - 1))
                            gs = slice(g * 512, (g + 1) * 512)
                            if h == 0:
                                nc.scalar.activation(unorm[:, d, gs], ps[:],
                                                     AF.Copy)
                            else:
                                nc.vector.tensor_tensor(unorm[:, d, gs],
                                                        unorm[:, d, gs], ps[:],
                                                        op=AOT.add)
                # pack rs_in
                for cb in range(NCORES):
                    for d in range(DT):
                        nc.sync.dma_start(
                            rs_in[cb, d * 128:(d + 1) * 128, :],
                            unorm[:, d, cb * RPC:(cb + 1) * RPC])
                zt = ps3t.tile([RT, 128], f32, tag="zt")
                nc.tensor.transpose(zt[:], z_cols[:], identf[:])
                zrow = p3.tile([RT, 128], f32, tag="zrow")
                nc.vector.tensor_copy(zrow[:], zt[:])
                zero = p3.tile([128, RPC], f32, tag="srl")
                nc.vector.memset(zero[:], 0.0)
                for cb in range(NCORES):
                    nc.sync.dma_start(
                        rs_in[cb, 1024:1025, :].rearrange(
                            "o (a b) -> (o a) b", a=2),
                        zrow[cb * 2:cb * 2 + 2, :])
                    nc.sync.dma_start(rs_in[cb, 1025:1152, :], zero[0:127, :])

        # ================= P4: causal attention (own block) ===============
        with tc.tile_pool(name="p4", bufs=1) as p4, \
             tc.tile_pool(name="p4b", bufs=2) as p4b, \
             tc.tile_pool(name="ps4", bufs=2, space="PSUM") as ps4, \
             tc.tile_pool(name="ps4s", bufs=2, space="PSUM") as ps4s, \
             tc.tile_pool(name="ps4p", bufs=2, space="PSUM") as ps4p, \
             tc.tile_pool(name="ps4t", bufs=2, space="PSUM") as ps4t:
            xb = p4.tile([128, DT, S], bf16, tag="xb")
            xqb = xq_hi
            wq = p4.tile([128, DT, D], bf16, tag="wq")
            wk = p4.tile([128, DT, D], bf16, tag="wk")
            wv = p4.tile([128, DT, D], bf16, tag="wv")
            wo = p4.tile([128, DT, D], bf16, tag="wo")
            vneg = p4.tile([128, 3], f32, tag="vneg")
            nc.sync.dma_start(vneg[:], VALIDN[:])
            nc.gpsimd.collective_compute(
                "ReduceScatter", AOT.add, replica_groups=RG,
                ins=[rs_in.opt()], outs=[rs_out.opt()])
            for src, dst in ((WQT, wq), (WKT, wk)):
                for d in range(DT):
                    tw = p4b.tile([128, D], f32, tag="wld", name="wld")
                    nc.sync.dma_start(tw[:], src[d * 128:(d + 1) * 128, :])
                    nc.vector.tensor_copy(dst[:, d, :], tw[:])
            for d in range(DT):
                t = p4b.tile([128, S], f32, tag="xld")
                nc.sync.dma_start(t[:], XT[d * 128:(d + 1) * 128, :])
                nc.vector.tensor_copy(xb[:, d, :], t[:])
            for src, dst in ((WVT, wv), (WOT, wo)):
                for d in range(DT):
                    tw = p4b.tile([128, D], f32, tag="wld", name="wld")
                    nc.sync.dma_start(tw[:], src[d * 128:(d + 1) * 128, :])
                    nc.vector.tensor_copy(dst[:, d, :], tw[:])
            qT = p4.tile([128, DT, RPC], bf16, tag="qT")
            kTp = p4.tile([128, DT, 768], bf16, tag="kTp")
            kTd = p4.tile([128, DT, RPC], bf16, tag="kTd")
            for d in range(DT):
                ps = ps4.tile([128, 512], f32, tag="pj")
                nc_d = slice(d * 128, (d + 1) * 128)
                for e in range(DT):
                    nc.tensor.matmul(ps[:, :RPC], wq[:, e, nc_d], xqb[:, e, :],
                                     start=(e == 0), stop=(e == DT - 1))
                nc.vector.tensor_copy(qT[:, d, :], ps[:, :RPC])
                for e in range(DT):
                    nc.tensor.matmul(ps[:, :RPC], wk[:, e, nc_d], xqb[:, e, :],
                                     start=(e == 0), stop=(e == DT - 1))
                nc.vector.tensor_copy(kTd[:, d, :], ps[:, :RPC])
                for chn in range(2):
                    cs = slice(chn * 512, min((chn + 1) * 512, 768))
                    w_ = cs.stop - cs.start
                    for e in range(DT):
                        nc.tensor.matmul(ps[:, :w_], wk[:, e, nc_d],
                                         xb[:, e, cs], start=(e == 0),
                                         stop=(e == DT - 1))
                    nc.vector.tensor_copy(kTp[:, d, cs], ps[:, :w_])
            vR = p4.tile([128, 6, D], bf16, tag="vR")
            vRd = p4.tile([128, MT, D], bf16, tag="vRd")
            for k in range(6):
                for chn in range(2):
                    ps = ps4.tile([128, 512], f32, tag="pj")
                    cs = slice(chn * 512, (chn + 1) * 512)
                    for e in range(DT):
                        nc.tensor.matmul(ps[:], xb[:, e, k * 128:(k + 1) * 128],
                                         wv[:, e, cs], start=(e == 0),
                                         stop=(e == DT - 1))
                    nc.vector.tensor_copy(vR[:, k, cs], ps[:])
            for m in range(MT):
                for chn in range(2):
                    ps = ps4.tile([128, 512], f32, tag="pj")
                    cs = slice(chn * 512, (chn + 1) * 512)
                    for e in range(DT):
                        nc.tensor.matmul(ps[:], xqb[:, e, m * 128:(m + 1) * 128],
                                         wv[:, e, cs], start=(e == 0),
                                         stop=(e == DT - 1))
                    nc.vector.tensor_copy(vRd[:, m, cs], ps[:])
            acT = p4.tile([128, DT, RPC], bf16, tag="acT")
            for hh in range(H):
                d, half = hh // 2, hh % 2
                hp = slice(64 * half, 64 * half + 64)
                for m in range(MT):
                    sc = p4b.tile([128, 1024], f32, tag="sc")
                    for chn in range(3):
                        ps = ps4s.tile([128, 256], f32, tag="sps")
                        nc.tensor.matmul(ps[:], qT[hp, d, m * 128:(m + 1) * 128],
                                         kTp[hp, d, chn * 256:(chn + 1) * 256])
                        nc.vector.tensor_scalar(
                            sc[:, chn * 256:(chn + 1) * 256], ps[:],
                            vneg[:, chn:chn + 1], scalar2=None, op0=AOT.add)
                    ps = ps4s.tile([128, 256], f32, tag="sps")
                    nc.tensor.matmul(ps[:], qT[hp, d, m * 128:(m + 1) * 128],
                                     kTd[hp, d, :])
                    nc.vector.tensor_tensor(sc[:, 768:1024], ps[:],
                                            dmask[:, m, :], op=AOT.add)
                    rm = p4b.tile([128, 1], f32, tag="rm")
                    nc.vector.tensor_reduce(rm[:], sc[:],
                                            axis=mybir.AxisListType.XYZW,
                                            op=AOT.max)
                    nc.vector.tensor_scalar_mul(rm[:], rm[:], -0.125)
                    ex = p4b.tile([128, 1024], f32, tag="ex")
                    zr = p4b.tile([128, 1], f32, tag="zr")
                    nc.scalar.activation(ex[:], sc[:], AF.Exp, bias=rm[:],
                                         scale=0.125, accum_out=zr[:])
                    iz = p4b.tile([128, 1], f32, tag="iz")
                    nc.vector.reciprocal(iz[:], zr[:])
                    ab = p4b.tile([128, 1024], bf16, tag="ab")
                    nc.scalar.activation(ab[:], ex[:], AF.Copy, scale=iz[:])
                    pv = ps4p.tile([128, 128], f32, tag="pv")
                    at = p4b.tile([128, 8, 128], bf16, tag="at")
                    for k in range(8):
                        tp = ps4t.tile([128, 128], bf16, tag="tp")
                        nc.tensor.transpose(tp[:], ab[:, k * 128:(k + 1) * 128],
                                            ident[:])
                        nc.vector.tensor_copy(at[:, k, :], tp[:])
                    for k in range(8):
                        vsrc = (vR[:, k, hh * 64:(hh + 1) * 64] if k < 6 else
                                vRd[:, k - 6, hh * 64:(hh + 1) * 64])
                        nc.tensor.matmul(pv[hp, :], vsrc, at[:, k, :],
                                         start=(k == 0), stop=(k == 7))
                    nc.vector.tensor_copy(acT[hp, d, m * 128:(m + 1) * 128],
                                          pv[hp, :])
            for d in range(DT):
                ps = ps4.tile([128, 512], f32, tag="pj")
                for e in range(DT):
                    nc.tensor.matmul(ps[:, :RPC], wo[:, e, d * 128:(d + 1) * 128],
                                     acT[:, e, :], start=(e == 0),
                                     stop=(e == DT - 1))
                nc.vector.tensor_copy(attn_oT[:, d, :], ps[:, :RPC])
                nc.vector.tensor_copy(attn_oTb[:, d, :], ps[:, :RPC])

        # ================= P5: finale =================
        with tc.tile_pool(name="p5", bufs=1) as p5, \
             tc.tile_pool(name="p5b", bufs=2) as p5b, \
             tc.tile_pool(name="ps5", bufs=2, space="PSUM") as ps5, \
             tc.tile_pool(name="ps5t", bufs=2, space="PSUM") as ps5t:
            knn_s = p5.tile([128, DT, RPC], f32, tag="kns")
            for d in range(DT):
                nc.sync.dma_start(knn_s[:, d, :],
                                  rs_out[d * 128:(d + 1) * 128, :])
            zg = p5.tile([1, RPC], f32, tag="zg")
            nc.sync.dma_start(zg[:], rs_out[1024:1025, :])
            iz = p5.tile([1, RPC], f32, tag="izg")
            nc.vector.reciprocal(iz[:], zg[:])
            ones_r = p5.tile([1, 128], f32, tag="ones")
            nc.vector.memset(ones_r[:], 1.0)
            bc = ps5.tile([128, RPC], f32, tag="bc")
            nc.tensor.matmul(bc[:], ones_r[:], iz[:])
            izb = p5.tile([128, RPC], f32, tag="izb")
            nc.vector.tensor_copy(izb[:], bc[:])
            knn_nb = p5.tile([128, DT, RPC], bf16, tag="knnb")
            for d in range(DT):
                nc.vector.tensor_tensor(knn_s[:, d, :], knn_s[:, d, :], izb[:],
                                        op=AOT.mult)
                nc.vector.tensor_copy(knn_nb[:, d, :], knn_s[:, d, :])
            wp = p5.tile([128, DT, D], bf16, tag="wp")
            bpr = p5.tile([128, DT], f32, tag="bpr")
            for d in range(DT):
                tw = p5b.tile([128, D], f32, tag="wpld")
                nc.sync.dma_start(tw[:], WPT[d * 128:(d + 1) * 128, :])
                nc.vector.tensor_copy(wp[:, d, :], tw[:])
                nc.sync.dma_start(bpr[:, d:d + 1],
                                  BPROJ[d * 128:(d + 1) * 128, :])
            knn_oT = p5.tile([128, DT, RPC], f32, tag="knoT")
            knn_oTb = p5.tile([128, DT, RPC], bf16, tag="knoTb")
            for d in range(DT):
                ps = ps5.tile([128, RPC], f32, tag="pmm")
                for e in range(DT):
                    nc.tensor.matmul(ps[:], wp[:, e, d * 128:(d + 1) * 128],
                                     knn_nb[:, e, :], start=(e == 0),
                                     stop=(e == DT - 1))
                nc.vector.tensor_scalar(knn_oT[:, d, :], ps[:],
                                        bpr[:, d:d + 1], scalar2=None,
                                        op0=AOT.add)
                nc.vector.tensor_copy(knn_oTb[:, d, :], knn_oT[:, d, :])
            wg = p5.tile([128, 2 * DT], bf16, tag="wg")
            wgf = p5.tile([128, 2 * DT], f32, tag="wgf")
            nc.sync.dma_start(wgf[:],
                              WGT[:].rearrange("(a p) 1 -> p a", p=128))
            nc.vector.tensor_copy(wg[:], wgf[:])
            bgt = p5.tile([1, 1], f32, tag="bgt")
            nc.sync.dma_start(bgt[:], BG[:])
            gps = ps5.tile([1, RPC], f32, tag="gps")
            for e in range(DT):
                nc.tensor.matmul(gps[:], wg[:, e:e + 1], attn_oTb[:, e, :],
                                 start=(e == 0), stop=False)
            for e in range(DT):
                nc.tensor.matmul(gps[:], wg[:, DT + e:DT + e + 1],
                                 knn_oTb[:, e, :], start=False,
                                 stop=(e == DT - 1))
            gate = p5.tile([1, RPC], f32, tag="gate")
            nc.scalar.activation(gate[:], gps[:], AF.Sigmoid, bias=bgt[0:1, 0:1])
            gbc = ps5.tile([128, RPC], f32, tag="bc")
            nc.tensor.matmul(gbc[:], ones_r[:], gate[:])
            gateb = p5.tile([128, RPC], f32, tag="gateb")
            nc.vector.tensor_copy(gateb[:], gbc[:])
            for d in range(DT):
                dif = p5b.tile([128, RPC], f32, tag="dif")
                nc.vector.tensor_tensor(dif[:], attn_oT[:, d, :],
                                        knn_oT[:, d, :], op=AOT.subtract)
                nc.vector.tensor_tensor(dif[:], dif[:], gateb[:], op=AOT.mult)
                nc.vector.tensor_tensor(knn_oT[:, d, :], knn_oT[:, d, :],
                                        dif[:], op=AOT.add)
            for m in range(MT):
                orow = p5b.tile([128, D], f32, tag="orow")
                for d in range(DT):
                    tp = ps5t.tile([128, 128], f32, tag="tp")
                    nc.tensor.transpose(tp[:],
                                        knn_oT[:, d, m * 128:(m + 1) * 128],
                                        identf[:])
                    nc.vector.tensor_copy(orow[:, d * 128:(d + 1) * 128], tp[:])
                nc.sync.dma_start(OUT[m * 128:(m + 1) * 128, :], orow[:])

    nc.compile()
    _BUILT["nc"] = nc
    return nc


def kernel(x, store_keys, store_vals, Wq, Wk, Wv, Wo, Wkk, Wproj, bproj,
           Wg, bg):
    import os
    x = np.asarray(x, np.float32)
    store_keys = np.asarray(store_keys, np.float32)
    store_vals = np.asarray(store_vals, np.float32)
    Wq, Wk, Wv, Wo, Wkk, Wproj = (np.asarray(w, np.float32)
                                  for w in (Wq, Wk, Wv, Wo, Wkk, Wproj))
    bproj = np.asarray(bproj, np.float32)
    Wg = np.asarray(Wg, np.float32)
    bg = np.asarray(bg, np.float32)

    nc = _build()

    xtb = [np.ascontiguousarray(x[b].T) for b in range(B)]
    wqt = np.ascontiguousarray(Wq.T)
    wkt = np.ascontiguousarray(Wk.T)
    wvt = np.ascontiguousarray(Wv.T)
    wot = np.ascontiguousarray(Wo.T)
    wkkt = np.ascontiguousarray(Wkk.T)
    wpt = np.ascontiguousarray(Wproj.T)
    wgt = np.ascontiguousarray(Wg[0].reshape(2 * D, 1))
    bgt = bg.reshape(1, 1)
    bprojc = np.ascontiguousarray(bproj.reshape(D, 1))

    in_maps = []
    for c in range(NCORES):
        b, blk = c // 4, c % 4
        q0 = blk * 256
        validn = np.zeros((128, 3), np.float32)
        for ch in range(3):
            if (ch + 1) * 256 > q0:
                validn[:, ch] = NEG
        in_maps.append({
            "xt": xtb[b],
            "xtq": np.ascontiguousarray(xtb[b][:, q0:q0 + RPC]),
            "keys": store_keys[c * SS:(c + 1) * SS],
            "vals": store_vals[c * SS:(c + 1) * SS],
            "wqt": wqt, "wkt": wkt, "wvt": wvt, "wot": wot,
            "wkkt": wkkt, "wpt": wpt, "wgt": wgt, "bg": bgt,
            "bproj": bprojc, "validn": validn,
        })

    from concourse.bass_utils import run_bass_kernel_spmd
    trace = bool(os.environ.get("KNN_TRACE"))
    res = run_bass_kernel_spmd(nc, in_maps, list(range(NCORES)), trace=trace,
                               tmpdir=os.environ.get("KNN_TRACE_DIR"))
    if trace:
        _BUILT["exec_time_ns"] = res.exec_time_ns
    out = np.concatenate([res.results[c]["out"] for c in range(NCORES)], axis=0)
    return out.reshape(B, S, D).astype(np.float32)


# revision 14
# speedup vs baseline: 1.1089x; 1.0005x over previous
"""KNN-attention layer on 8 Trainium2 NeuronCores (Bass/Tile).

Sharding: core c owns query rows [256c, 256c+256) (batch c//4) and store
shard [4096c, 4096(c+1)).

Per-core program (static, identical on all cores):
  P0  knn-query projection (bf16 hi/lo split-3 matmul for fp32-exact sims),
      normalize, transpose, AllGather qn across cores; build diag causal masks.
  P1  per store half (2048): normalize+split+transpose keys; sims = qn @ kn.T
      for all 2048 rows via split-3; spill sims to DRAM; local top-32 per half
      (max8 + match_replace); in half 1, merge halves and AllGather candidates.
  P2  global threshold t (32nd) and max m per row from gathered candidates.
  P3  per half: reload sims, W = (s >= t) * exp((s-m)/T) in bf16 (+Z accum),
      transpose W, weighted matmul with store_vals -> unnormalized knn partial
      [1024, 2048]; pack + ReduceScatter (sum over cores, scatter by row block).
  P4  causal self-attention for own row block (bf16), static chunking:
      3 absolute 256-wide key pre-chunks gated by a per-core validity column
      + 1 diagonal chunk from the own-block x slice. Overlaps the RS (no
      gpsimd ops here; collectives own the gpsimd queue).
  P5  normalize knn by Z, project (+bias), gated combine, transpose to
      row-major, output [256, 1024] per core.

Selection precision: sims are computed as q_hi@k_hi + q_hi@k_lo + q_lo@k_hi
(bf16 splits, fp32 PSUM accumulation) which matches fp32 sims to ~1e-7 —
required because the global top-32 set must match the fp32 reference's.
"""
import sys

sys.path.insert(0, "/opt/trn_rl_repo")

import numpy as np

# ---------------- problem constants ----------------
B, S, D = 2, 1024, 1024
H, HD = 16, 64
N = 32768
KNN_K = 32
TEMP = 0.1
NCORES = 8
ROWS = B * S                # 2048
RPC = ROWS // NCORES        # 256 rows per core
SS = N // NCORES            # 4096 stores per core
HNS = SS // 2               # 2048 per half
DT = D // 128               # 8 feature tiles
RT = ROWS // 128            # 16 global row tiles
MT = RPC // 128             # 2 own row tiles
HIT = HNS // 128            # 16 store tiles per half
NEG = -1e30
RG = [list(range(NCORES))]

_BUILT = {}


def _build():
    if "nc" in _BUILT:
        return _BUILT["nc"]
    from contextlib import ExitStack
    import concourse.bass as bass
    import concourse.tile as tile
    from concourse import bacc, mybir
    from concourse.masks import make_identity

    f32 = mybir.dt.float32
    bf16 = mybir.dt.bfloat16
    AOT = mybir.AluOpType
    AF = mybir.ActivationFunctionType

    nc = bacc.Bacc("TRN2", target_bir_lowering=False, debug=False,
                   num_devices=NCORES)

    # ---- I/O ----
    XT = nc.dram_tensor("xt", [D, S], f32, kind="ExternalInput").ap()
    XTQ = nc.dram_tensor("xtq", [D, RPC], f32, kind="ExternalInput").ap()
    KEYS = nc.dram_tensor("keys", [SS, D], f32, kind="ExternalInput").ap()
    VALS = nc.dram_tensor("vals", [SS, D], f32, kind="ExternalInput").ap()
    WQT = nc.dram_tensor("wqt", [D, D], f32, kind="ExternalInput").ap()
    WKT = nc.dram_tensor("wkt", [D, D], f32, kind="ExternalInput").ap()
    WVT = nc.dram_tensor("wvt", [D, D], f32, kind="ExternalInput").ap()
    WOT = nc.dram_tensor("wot", [D, D], f32, kind="ExternalInput").ap()
    WKKT = nc.dram_tensor("wkkt", [D, D], f32, kind="ExternalInput").ap()
    WPT = nc.dram_tensor("wpt", [D, D], f32, kind="ExternalInput").ap()
    WGT = nc.dram_tensor("wgt", [2 * D, 1], f32, kind="ExternalInput").ap()
    BG = nc.dram_tensor("bg", [1, 1], f32, kind="ExternalInput").ap()
    BPROJ = nc.dram_tensor("bproj", [D, 1], f32, kind="ExternalInput").ap()
    VALIDN = nc.dram_tensor("validn", [128, 3], f32, kind="ExternalInput").ap()
    OUT = nc.dram_tensor("out", [RPC, D], f32, kind="ExternalOutput").ap()

    # ---- DRAM scratch ----
    simsbuf = nc.dram_tensor("simsbuf", [RT, 2, 128, HNS], f32,
                             kind="Internal").ap()
    qn_b = nc.dram_tensor("qn_b", [2, D, RPC], bf16, kind="Internal").ap()
    qn_g = nc.dram_tensor("qn_g", [NCORES, 2, D, RPC], bf16, kind="Internal",
                          addr_space="Shared").ap()
    cand_b = nc.dram_tensor("cand_b", [RT, 128, 24], f32, kind="Internal").ap()
    cand_g = nc.dram_tensor("cand_g", [NCORES, RT, 128, 24], f32,
                            kind="Internal", addr_space="Shared").ap()
    rs_in = nc.dram_tensor("rs_in", [NCORES, 1152, RPC], f32,
                           kind="Internal").ap()
    rs_out = nc.dram_tensor("rs_out", [1152, RPC], f32, kind="Internal").ap()

    def rsqrt_newton(pool, n2, tagp):
        r0 = pool.tile([128, 1], f32, tag=tagp + "r0", name=tagp + "r0")
        nc.vector.reciprocal(r0[:], n2)
        y = pool.tile([128, 1], f32, tag=tagp + "y", name=tagp + "y")
        nc.scalar.activation(y[:], r0[:], AF.Sqrt)
        for it in ("a", "b"):
            yy = pool.tile([128, 1], f32, tag=tagp + it + "1",
                           name=tagp + it + "1")
            nc.vector.tensor_tensor(yy[:], y[:], y[:], op=AOT.mult)
            nc.vector.tensor_tensor(yy[:], n2, yy[:], op=AOT.mult)
            nc.vector.tensor_scalar(yy[:], yy[:], -0.5, scalar2=1.5,
                                    op0=AOT.mult, op1=AOT.add)
            y2 = pool.tile([128, 1], f32, tag=tagp + it + "2",
                           name=tagp + it + "2")
            nc.vector.tensor_tensor(y2[:], y[:], yy[:], op=AOT.mult)
            y = y2
        return y

    def split_hi_lo(src, hi, lo):
        nc.vector.tensor_copy(hi, src)
        nc.vector.tensor_tensor(lo, src, hi, op=AOT.subtract)

    with tile.TileContext(nc) as tc, ExitStack() as ctx:
        res = ctx.enter_context(tc.tile_pool(name="res", bufs=1))
        ident = res.tile([128, 128], bf16)
        make_identity(nc, ident)
        identf = res.tile([128, 128], f32)
        make_identity(nc, identf)
        maxes_all = res.tile([128, RT, 48], f32)
        t_m = res.tile([128, RT, 2], f32)
        attn_oT = res.tile([128, DT, RPC], f32)
        attn_oTb = res.tile([128, DT, RPC], bf16)
        dmask = res.tile([128, MT, 256], f32)

        # ================= P0: qn + AllGather + diag masks ================
        with tc.tile_pool(name="p0", bufs=2) as p0, \
             tc.tile_pool(name="ps0", bufs=2, space="PSUM") as ps0, \
             tc.tile_pool(name="ps0t", bufs=2, space="PSUM") as ps0t:
            # diagonal causal masks (additive 0/NEG), before any collective
            nc.vector.memset(dmask[:], 0.0)
            for m in range(MT):
                nc.gpsimd.affine_select(
                    dmask[:, m, :], dmask[:, m, :],
                    pattern=[[-1, 256]], compare_op=AOT.is_ge, fill=NEG,
                    base=m * 128, channel_multiplier=1)
            xq_hi = res.tile([128, DT, RPC], bf16)
            xq_lo = p0.tile([128, DT, RPC], bf16, tag="xql")
            wk_hi = p0.tile([128, DT, D], bf16, tag="wkh")
            wk_lo = p0.tile([128, DT, D], bf16, tag="wkl")
            for d in range(DT):
                t = p0.tile([128, RPC], f32, tag="ld")
                nc.sync.dma_start(t[:], XTQ[d * 128:(d + 1) * 128, :])
                split_hi_lo(t[:], xq_hi[:, d, :], xq_lo[:, d, :])
                tw = p0.tile([128, D], f32, tag="ldw")
                nc.sync.dma_start(tw[:], WKKT[d * 128:(d + 1) * 128, :])
                split_hi_lo(tw[:], wk_hi[:, d, :], wk_lo[:, d, :])
            for m in range(MT):
                qrow = p0.tile([128, D], f32, tag="qrow")
                for chn in range(2):
                    ps = ps0.tile([128, 512], f32, tag="mm")
                    cs = slice(chn * 512, (chn + 1) * 512)
                    ms = slice(m * 128, (m + 1) * 128)
                    for d in range(DT):
                        nc.tensor.matmul(ps[:], xq_hi[:, d, ms], wk_hi[:, d, cs],
                                         start=(d == 0), stop=False)
                        nc.tensor.matmul(ps[:], xq_hi[:, d, ms], wk_lo[:, d, cs],
                                         start=False, stop=False)
                        nc.tensor.matmul(ps[:], xq_lo[:, d, ms], wk_hi[:, d, cs],
                                         start=False, stop=(d == DT - 1))
                    nc.vector.tensor_copy(qrow[:, cs], ps[:])
                n2 = p0.tile([128, 1], f32, tag="n2")
                sq = p0.tile([128, D], f32, tag="sq")
                nc.scalar.activation(sq[:], qrow[:], AF.Square, accum_out=n2[:])
                inv = rsqrt_newton(p0, n2[:], "nq")
                nc.vector.tensor_scalar_mul(qrow[:], qrow[:], inv[:])
                q_hi = p0.tile([128, D], bf16, tag="qhi")
                q_lo = p0.tile([128, D], bf16, tag="qlo")
                split_hi_lo(qrow[:], q_hi[:], q_lo[:])
                for d in range(DT):
                    for hl, src in enumerate((q_hi, q_lo)):
                        tp = ps0t.tile([128, 128], bf16, tag="tp")
                        nc.tensor.transpose(tp[:], src[:, d * 128:(d + 1) * 128],
                                            ident[:])
                        sb = p0.tile([128, 128], bf16, tag="tpo")
                        nc.vector.tensor_copy(sb[:], tp[:])
                        nc.sync.dma_start(
                            qn_b[hl, d * 128:(d + 1) * 128,
                                 m * 128:(m + 1) * 128], sb[:])
            nc.gpsimd.collective_compute(
                "AllGather", AOT.bypass, replica_groups=RG,
                ins=[qn_b.opt()], outs=[qn_g.opt()])

        # ============ P1: keys + sims + local topk (both halves) ==========
        def keys_half(h, p1, p1b, p1c, ps1t, knT_hi, knT_lo):
            for i in range(HIT):
                kt = p1b.tile([128, D], f32, tag="kld", name="kld")
                nc.sync.dma_start(
                    kt[:], KEYS[h * HNS + i * 128:h * HNS + (i + 1) * 128, :])
                n2 = p1c.tile([128, 1], f32, tag="kn2", name="kn2")
                sq = p1c.tile([128, D], f32, tag="scr", name="ksq")
                nc.scalar.activation(sq[:], kt[:], AF.Square, accum_out=n2[:])
                inv = rsqrt_newton(p1c, n2[:], "nk")
                nc.vector.tensor_scalar_mul(kt[:], kt[:], inv[:])
                k_hi = p1b.tile([128, D], bf16, tag="khi", name="khi")
                k_lo = p1b.tile([128, D], bf16, tag="klo", name="klo")
                split_hi_lo(kt[:], k_hi[:], k_lo[:])
                for d in range(DT):
                    for src, dst in ((k_hi, knT_hi), (k_lo, knT_lo)):
                        tp = ps1t.tile([128, 128], bf16, tag="tp", name="tp")
                        nc.tensor.transpose(
                            tp[:], src[:, d * 128:(d + 1) * 128], ident[:])
                        if d % 2 == 0:
                            nc.scalar.activation(
                                dst[:, d, i * 128:(i + 1) * 128], tp[:], AF.Copy)
                        else:
                            nc.vector.tensor_copy(
                                dst[:, d, i * 128:(i + 1) * 128], tp[:])

        def sims_half(h, p1b, p1c, ps1, knT_hi, knT_lo, merge_local):
            for rt in range(RT):
                cb, ml = rt // 2, rt % 2
                qh = p1b.tile([128, DT, 128], bf16, tag="qsh", name="qsh")
                ql = p1b.tile([128, DT, 128], bf16, tag="qsl", name="qsl")
                for hl, dst in ((0, qh), (1, ql)):
                    nc.sync.dma_start(
                        dst[:],
                        qn_g[cb, hl, :, ml * 128:(ml + 1) * 128].rearrange(
                            "(dt p) c -> p dt c", p=128))
                sims = p1b.tile([128, HNS], f32, tag="sims", name="sims")
                pss = [ps1.tile([128, 512], f32, tag=f"mm{i}",
                                name=f"mm{i}", bufs=(2 if i < 2 else 1))
                       for i in range(4)]
                for d in range(DT):
                    for chn in range(4):
                        nc.tensor.matmul(pss[chn][:], qh[:, d, :],
                                         knT_hi[:, d, chn * 512:(chn + 1) * 512],
                                         start=(d == 0), stop=False)
                    for chn in range(4):
                        nc.tensor.matmul(pss[chn][:], qh[:, d, :],
                                         knT_lo[:, d, chn * 512:(chn + 1) * 512],
                                         start=False, stop=False)
                    for chn in range(4):
                        nc.tensor.matmul(pss[chn][:], ql[:, d, :],
                                         knT_hi[:, d, chn * 512:(chn + 1) * 512],
                                         start=False, stop=(d == DT - 1))
                for chn in range(4):
                    nc.scalar.activation(sims[:, chn * 512:(chn + 1) * 512],
                                         pss[chn][:], AF.Copy)
                nc.sync.dma_start(simsbuf[rt, h], sims[:])
                scr = p1c.tile([128, HNS], f32, tag="scr", name="scr")
                nc.vector.tensor_copy(scr[:], sims[:])
                for r in range(3):
                    mx = maxes_all[:, rt, h * 24 + r * 8:h * 24 + (r + 1) * 8]
                    nc.vector.max(mx, scr[:])
                    nc.vector.match_replace(scr[:], mx, scr[:], NEG)
                if merge_local:
                    mscr = p1c.tile([128, 48], f32, tag="mscr", name="mscr")
                    nc.vector.tensor_copy(mscr[:], maxes_all[:, rt, :])
                    loc = p1c.tile([128, 24], f32, tag="loc", name="loc")
                    for r in range(3):
                        nc.vector.max(loc[:, r * 8:(r + 1) * 8], mscr[:])
                        nc.vector.match_replace(mscr[:], loc[:, r * 8:(r + 1) * 8],
                                                mscr[:], NEG)
                    nc.sync.dma_start(cand_b[rt], loc[:])

        with tc.tile_pool(name="vp", bufs=2) as vpool:
            vals_bf0 = vpool.tile([128, HIT, D], bf16, tag="vals", name="vals0")
            for i in range(HIT):
                vt = vpool.tile([128, D], f32, tag="vld", name="vld")
                nc.sync.dma_start(vt[:], VALS[i * 128:(i + 1) * 128, :])
                nc.vector.tensor_copy(vals_bf0[:, i, :], vt[:])
            for h in range(2):
                with tc.tile_pool(name=f"p1_{h}", bufs=1) as p1, \
                     tc.tile_pool(name=f"p1b_{h}", bufs=2) as p1b, \
                     tc.tile_pool(name=f"p1c_{h}", bufs=1) as p1c, \
                     tc.tile_pool(name=f"ps1_{h}", bufs=1, space="PSUM") as ps1, \
                     tc.tile_pool(name=f"ps1t_{h}", bufs=2, space="PSUM") as ps1t:
                    knT_hi = p1.tile([128, DT, HNS], bf16, tag="knh", name="knh")
                    knT_lo = p1.tile([128, DT, HNS], bf16, tag="knl", name="knl")
                    keys_half(h, p1, p1b, p1c, ps1t, knT_hi, knT_lo)
                    sims_half(h, p1b, p1c, ps1, knT_hi, knT_lo, h == 1)
            nc.gpsimd.collective_compute(
                "AllGather", AOT.bypass, replica_groups=RG,
                ins=[cand_b.opt()], outs=[cand_g.opt()])

            # ============ P3: W, weighted matmul (merge inlined) ============
            with tc.tile_pool(name="p3", bufs=1) as p3, \
                 tc.tile_pool(name="p3b", bufs=2) as p3b, \
                 tc.tile_pool(name="ps3", bufs=4, space="PSUM") as ps3, \
                 tc.tile_pool(name="ps3t", bufs=2, space="PSUM") as ps3t:
                unorm = p3.tile([128, DT, ROWS], f32, tag="unorm")
                z_cols = p3.tile([128, RT], f32, tag="zc")
                for h in range(2):
                    if h == 0:
                        vals_bf = vals_bf0
                    else:
                        vals_bf = vpool.tile([128, HIT, D], bf16, tag="vals",
                                             name="vals1")
                        for i in range(HIT):
                            vt = vpool.tile([128, D], f32, tag="vld", name="vld")
                            nc.sync.dma_start(
                                vt[:],
                                VALS[HNS + i * 128:HNS + (i + 1) * 128, :])
                            nc.vector.tensor_copy(vals_bf[:, i, :], vt[:])
                    for g in range(4):
                        w_T = p3.tile([128, HIT, 512], bf16, tag="wT",
                                      name="wT")
                        for rl in range(4):
                            rt = g * 4 + rl
                            if h == 0:
                                gall = p3.tile([128, NCORES * 24], f32,
                                               tag="gall", name="gall")
                                for cb in range(NCORES):
                                    nc.sync.dma_start(
                                        gall[:, cb * 24:(cb + 1) * 24],
                                        cand_g[cb, rt])
                                gm = p3.tile([128, 4, 8], f32, tag="gm",
                                             name="gm")
                                for r in range(4):
                                    nc.vector.max(gm[:, r, :], gall[:])
                                    nc.vector.match_replace(gall[:], gm[:, r, :],
                                                            gall[:], NEG)
                                nc.vector.tensor_copy(t_m[:, rt, 0:1],
                                                      gm[:, 3, 7:8])
                                nc.vector.tensor_copy(t_m[:, rt, 1:2],
                                                      gm[:, 0, 0:1])
                            sims = p3.tile([128, HNS], f32, tag="srl",
                                           name="srl")
                            nc.sync.dma_start(sims[:], simsbuf[rt, h])
                            mbias = p3b.tile([128, 1], f32, tag="mb", name="mb")
                            nc.vector.tensor_scalar_mul(mbias[:],
                                                        t_m[:, rt, 1:2],
                                                        -1.0 / TEMP)
                            expw = p3b.tile([128, HNS], f32, tag="expw",
                                            name="expw")
                            nc.scalar.activation(expw[:], sims[:], AF.Exp,
                                                 bias=mbias[:], scale=1.0 / TEMP)
                            wmask = p3b.tile([128, HNS], bf16, tag="wm",
                                             name="wm")
                            z = p3b.tile([128, 1], f32, tag="z", name="z")
                            nc.vector.scalar_tensor_tensor(
                                wmask[:], sims[:], t_m[:, rt, 0:1], expw[:],
                                op0=AOT.is_ge, op1=AOT.mult, accum_out=z[:])
                            if h == 0:
                                nc.vector.tensor_copy(z_cols[:, rt:rt + 1], z[:])
                            else:
                                nc.vector.tensor_tensor(
                                    z_cols[:, rt:rt + 1], z_cols[:, rt:rt + 1],
                                    z[:], op=AOT.add)
                            for i in range(HIT):
                                tp = ps3t.tile([128, 128], bf16, tag="tp",
                                               name="tp")
                                nc.tensor.transpose(
                                    tp[:], wmask[:, i * 128:(i + 1) * 128],
                                    ident[:])
                                if i % 2 == 0:
                                    nc.scalar.activation(
                                        w_T[:, i, rl * 128:(rl + 1) * 128],
                                        tp[:], AF.Copy)
                                else:
                                    nc.vector.tensor_copy(
                                        w_T[:, i, rl * 128:(rl + 1) * 128],
                                        tp[:])
                        for d in range(DT):
                            ps = ps3.tile([128, 512], f32, tag="mm", name="mm")
                            for i in range(HIT):
                                nc.tensor.matmul(
                                    ps[:], vals_bf[:, i, d * 128:(d + 1) * 128],
                                    w_T[:, i, :], start=(i == 0),
                                    stop=(i == HIT - 1))
                            gs = slice(g * 512, (g + 1) * 512)
                            if h == 0:
                                nc.scalar.activation(unorm[:, d, gs], ps[:],
                                                     AF.Copy)
                            else:
                                nc.vector.tensor_tensor(unorm[:, d, gs],
                                                        unorm[:, d, gs], ps[:],
                                                        op=AOT.add)
                # pack rs_in
                for cb in range(NCORES):
                    for d in range(DT):
                        nc.sync.dma_start(
                            rs_in[cb, d * 128:(d + 1) * 128, :],
                            unorm[:, d, cb * RPC:(cb + 1) * RPC])
                zt = ps3t.tile([RT, 128], f32, tag="zt")
                nc.tensor.transpose(zt[:], z_cols[:], identf[:])
                zrow = p3.tile([RT, 128], f32, tag="zrow")
                nc.vector.tensor_copy(zrow[:], zt[:])
                zero = p3.tile([128, RPC], f32, tag="srl")
                nc.vector.memset(zero[:], 0.0)
                for cb in range(NCORES):
                    nc.sync.dma_start(
                        rs_in[cb, 1024:1025, :].rearrange(
                            "o (a b) -> (o a) b", a=2),
                        zrow[cb * 2:cb * 2 + 2, :])
                    nc.sync.dma_start(rs_in[cb, 1025:1152, :], zero[0:127, :])

        # ================= P4: causal attention (own block) ===============
        with tc.tile_pool(name="p4", bufs=1) as p4, \
             tc.tile_pool(name="p4b", bufs=2) as p4b, \
             tc.tile_pool(name="ps4", bufs=2, space="PSUM") as ps4, \
             tc.tile_pool(name="ps4s", bufs=2, space="PSUM") as ps4s, \
             tc.tile_pool(name="ps4p", bufs=2, space="PSUM") as ps4p, \
             tc.tile_pool(name="ps4t", bufs=2, space="PSUM") as ps4t:
            xb = p4.tile([128, DT, S], bf16, tag="xb")
            xqb = xq_hi
            wq = p4.tile([128, DT, D], bf16, tag="wq")
            wk = p4.tile([128, DT, D], bf16, tag="wk")
            wv = p4.tile([128, DT, D], bf16, tag="wv")
            wo = p4.tile([128, DT, D], bf16, tag="wo")
            vneg = p4.tile([128, 3], f32, tag="vneg")
            nc.sync.dma_start(vneg[:], VALIDN[:])
            nc.gpsimd.collective_compute(
                "ReduceScatter", AOT.add, replica_groups=RG,
                ins=[rs_in.opt()], outs=[rs_out.opt()])
            for src, dst in ((WQT, wq), (WKT, wk)):
                for d in range(DT):
                    tw = p4b.tile([128, D], f32, tag="wld", name="wld")
                    nc.sync.dma_start(tw[:], src[d * 128:(d + 1) * 128, :])
                    nc.vector.tensor_copy(dst[:, d, :], tw[:])
            for d in range(DT):
                t = p4b.tile([128, S], f32, tag="xld")
                nc.sync.dma_start(t[:], XT[d * 128:(d + 1) * 128, :])
                nc.vector.tensor_copy(xb[:, d, :], t[:])
            for src, dst in ((WVT, wv), (WOT, wo)):
                for d in range(DT):
                    tw = p4b.tile([128, D], f32, tag="wld", name="wld")
                    nc.sync.dma_start(tw[:], src[d * 128:(d + 1) * 128, :])
                    nc.vector.tensor_copy(dst[:, d, :], tw[:])
            qT = p4.tile([128, DT, RPC], bf16, tag="qT")
            kTp = p4.tile([128, DT, 768], bf16, tag="kTp")
            kTd = p4.tile([128, DT, RPC], bf16, tag="kTd")
            for d in range(DT):
                ps = ps4.tile([128, 512], f32, tag="pj")
                nc_d = slice(d * 128, (d + 1) * 128)
                for e in range(DT):
                    nc.tensor.matmul(ps[:, :RPC], wq[:, e, nc_d], xqb[:, e, :],
                                     start=(e == 0), stop=(e == DT - 1))
                nc.vector.tensor_copy(qT[:, d, :], ps[:, :RPC])
                for e in range(DT):
                    nc.tensor.matmul(ps[:, :RPC], wk[:, e, nc_d], xqb[:, e, :],
                                     start=(e == 0), stop=(e == DT - 1))
                nc.vector.tensor_copy(kTd[:, d, :], ps[:, :RPC])
                for chn in range(2):
                    cs = slice(chn * 512, min((chn + 1) * 512, 768))
                    w_ = cs.stop - cs.start
                    for e in range(DT):
                        nc.tensor.matmul(ps[:, :w_], wk[:, e, nc_d],
                                         xb[:, e, cs], start=(e == 0),
                                         stop=(e == DT - 1))
                    nc.vector.tensor_copy(kTp[:, d, cs], ps[:, :w_])
            vR = p4.tile([128, 6, D], bf16, tag="vR")
            vRd = p4.tile([128, MT, D], bf16, tag="vRd")
            for k in range(6):
                for chn in range(2):
                    ps = ps4.tile([128, 512], f32, tag="pj")
                    cs = slice(chn * 512, (chn + 1) * 512)
                    for e in range(DT):
                        nc.tensor.matmul(ps[:], xb[:, e, k * 128:(k + 1) * 128],
                                         wv[:, e, cs], start=(e == 0),
                                         stop=(e == DT - 1))
                    nc.vector.tensor_copy(vR[:, k, cs], ps[:])
            for m in range(MT):
                for chn in range(2):
                    ps = ps4.tile([128, 512], f32, tag="pj")
                    cs = slice(chn * 512, (chn + 1) * 512)
                    for e in range(DT):
                        nc.tensor.matmul(ps[:], xqb[:, e, m * 128:(m + 1) * 128],
                                         wv[:, e, cs], start=(e == 0),
                                         stop=(e == DT - 1))
                    nc.vector.tensor_copy(vRd[:, m, cs], ps[:])
            acT = p4.tile([128, DT, RPC], bf16, tag="acT")
            for hh in range(H):
                d, half = hh // 2, hh % 2
                hp = slice(64 * half, 64 * half + 64)
                for m in range(MT):
                    sc = p4b.tile([128, 1024], f32, tag="sc")
                    for chn in range(3):
                        ps = ps4s.tile([128, 256], f32, tag="sps")
                        nc.tensor.matmul(ps[:], qT[hp, d, m * 128:(m + 1) * 128],
                                         kTp[hp, d, chn * 256:(chn + 1) * 256])
                        nc.vector.tensor_scalar(
                            sc[:, chn * 256:(chn + 1) * 256], ps[:],
                            vneg[:, chn:chn + 1], scalar2=None, op0=AOT.add)
                    ps = ps4s.tile([128, 256], f32, tag="sps")
                    nc.tensor.matmul(ps[:], qT[hp, d, m * 128:(m + 1) * 128],
                                     kTd[hp, d, :])
                    nc.vector.tensor_tensor(sc[:, 768:1024], ps[:],
                                            dmask[:, m, :], op=AOT.add)
                    rm = p4b.tile([128, 1], f32, tag="rm")
                    nc.vector.tensor_reduce(rm[:], sc[:],
                                            axis=mybir.AxisListType.XYZW,
                                            op=AOT.max)
                    nc.vector.tensor_scalar_mul(rm[:], rm[:], -0.125)
                    ex = p4b.tile([128, 1024], f32, tag="ex")
                    zr = p4b.tile([128, 1], f32, tag="zr")
                    nc.scalar.activation(ex[:], sc[:], AF.Exp, bias=rm[:],
                                         scale=0.125, accum_out=zr[:])
                    iz = p4b.tile([128, 1], f32, tag="iz")
                    nc.vector.reciprocal(iz[:], zr[:])
                    ab = p4b.tile([128, 1024], bf16, tag="ab")
                    nc.scalar.activation(ab[:], ex[:], AF.Copy, scale=iz[:])
                    pv = ps4p.tile([128, 128], f32, tag="pv")
                    at = p4b.tile([128, 8, 128], bf16, tag="at")
                    for k in range(8):
                        tp = ps4t.tile([128, 128], bf16, tag="tp")
                        nc.tensor.transpose(tp[:], ab[:, k * 128:(k + 1) * 128],
                                            ident[:])
                        nc.vector.tensor_copy(at[:, k, :], tp[:])
                    for k in range(8):
                        vsrc = (vR[:, k, hh * 64:(hh + 1) * 64] if k < 6 else
                                vRd[:, k - 6, hh * 64:(hh + 1) * 64])
                        nc.tensor.matmul(pv[hp, :], vsrc, at[:, k, :],
                                         start=(k == 0), stop=(k == 7))
                    nc.vector.tensor_copy(acT[hp, d, m * 128:(m + 1) * 128],
                                          pv[hp, :])
            for d in range(DT):
                ps = ps4.tile([128, 512], f32, tag="pj")
                for e in range(DT):
                    nc.tensor.matmul(ps[:, :RPC], wo[:, e, d * 128:(d + 1) * 128],
                                     acT[:, e, :], start=(e == 0),
                                     stop=(e == DT - 1))
                nc.vector.tensor_copy(attn_oT[:, d, :], ps[:, :RPC])
                nc.vector.tensor_copy(attn_oTb[:, d, :], ps[:, :RPC])

        # ================= P5: finale =================
        with tc.tile_pool(name="p5", bufs=1) as p5, \
             tc.tile_pool(name="p5b", bufs=2) as p5b, \
             tc.tile_pool(name="ps5", bufs=2, space="PSUM") as ps5, \
             tc.tile_pool(name="ps5t", bufs=2, space="PSUM") as ps5t:
            knn_s = p5.tile([128, DT, RPC], f32, tag="kns")
            for d in range(DT):
                nc.sync.dma_start(knn_s[:, d, :],
                                  rs_out[d * 128:(d + 1) * 128, :])
            zg = p5.tile([1, RPC], f32, tag="zg")
            nc.sync.dma_start(zg[:], rs_out[1024:1025, :])
            iz = p5.tile([1, RPC], f32, tag="izg")
            nc.vector.reciprocal(iz[:], zg[:])
            ones_r = p5.tile([1, 128], f32, tag="ones")
            nc.vector.memset(ones_r[:], 1.0)
            bc = ps5.tile([128, RPC], f32, tag="bc")
            nc.tensor.matmul(bc[:], ones_r[:], iz[:])
            izb = p5.tile([128, RPC], f32, tag="izb")
            nc.vector.tensor_copy(izb[:], bc[:])
            knn_nb = p5.tile([128, DT, RPC], bf16, tag="knnb")
            for d in range(DT):
                nc.vector.tensor_tensor(knn_s[:, d, :], knn_s[:, d, :], izb[:],
                                        op=AOT.mult)
                nc.vector.tensor_copy(knn_nb[:, d, :], knn_s[:, d, :])
            wp = p5.tile([128, DT, D], bf16, tag="wp")
            bpr = p5.tile([128, DT], f32, tag="bpr")
            for d in range(DT):
                tw = p5b.tile([128, D], f32, tag="wpld")
                nc.sync.dma_start(tw[:], WPT[d * 128:(d + 1) * 128, :])
                nc.vector.tensor_copy(wp[:, d, :], tw[:])
                nc.sync.dma_start(bpr[:, d:d + 1],
                                  BPROJ[d * 128:(d + 1) * 128, :])
            knn_oT = p5.tile([128, DT, RPC], f32, tag="knoT")
            knn_oTb = p5.tile([128, DT, RPC], bf16, tag="knoTb")
            for d in range(DT):
                ps = ps5.tile([128, RPC], f32, tag="pmm")
                for e in range(DT):
                    nc.tensor.matmul(ps[:], wp[:, e, d * 128:(d + 1) * 128],
                                     knn_nb[:, e, :], start=(e == 0),
                                     stop=(e == DT - 1))
                nc.vector.tensor_scalar(knn_oT[:, d, :], ps[:],
                                        bpr[:, d:d + 1], scalar2=None,
                                        op0=AOT.add)
                nc.vector.tensor_copy(knn_oTb[:, d, :], knn_oT[:, d, :])
            wg = p5.tile([128, 2 * DT], bf16, tag="wg")
            wgf = p5.tile([128, 2 * DT], f32, tag="wgf")
            nc.sync.dma_start(wgf[:],
                              WGT[:].rearrange("(a p) 1 -> p a", p=128))
            nc.vector.tensor_copy(wg[:], wgf[:])
            bgt = p5.tile([1, 1], f32, tag="bgt")
            nc.sync.dma_start(bgt[:], BG[:])
            gps = ps5.tile([1, RPC], f32, tag="gps")
            for e in range(DT):
                nc.tensor.matmul(gps[:], wg[:, e:e + 1], attn_oTb[:, e, :],
                                 start=(e == 0), stop=False)
            for e in range(DT):
                nc.tensor.matmul(gps[:], wg[:, DT + e:DT + e + 1],
                                 knn_oTb[:, e, :], start=False,
                                 stop=(e == DT - 1))
            gate = p5.tile([1, RPC], f32, tag="gate")
            nc.scalar.activation(gate[:], gps[:], AF.Sigmoid, bias=bgt[0:1, 0:1])
            gbc = ps5.tile([128, RPC], f32, tag="bc")
            nc.tensor.matmul(gbc[:], ones_r[:], gate[:])
            gateb = p5.tile([128, RPC], f32, tag="gateb")
            nc.vector.tensor_copy(gateb[:], gbc[:])
            for d in range(DT):
                dif = p5b.tile([128, RPC], f32, tag="dif")
                nc.vector.tensor_tensor(dif[:], attn_oT[:, d, :],
                                        knn_oT[:, d, :], op=AOT.subtract)
                nc.vector.tensor_tensor(dif[:], dif[:], gateb[:], op=AOT.mult)
                nc.vector.tensor_tensor(knn_oT[:, d, :], knn_oT[:, d, :],
                                        dif[:], op=AOT.add)
            for m in range(MT):
                orow = p5b.tile([128, D], f32, tag="orow")
                for d in range(DT):
                    tp = ps5t.tile([128, 128], f32, tag="tp")
                    nc.tensor.transpose(tp[:],
                                        knn_oT[:, d, m * 128:(m + 1) * 128],
                                        identf[:])
                    nc.vector.tensor_copy(orow[:, d * 128:(d + 1) * 128], tp[:])
                nc.sync.dma_start(OUT[m * 128:(m + 1) * 128, :], orow[:])

    nc.compile()
    _BUILT["nc"] = nc
    return nc


def kernel(x, store_keys, store_vals, Wq, Wk, Wv, Wo, Wkk, Wproj, bproj,
           Wg, bg):
    import os
    x = np.asarray(x, np.float32)
    store_keys = np.asarray(store_keys, np.float32)
    store_vals = np.asarray(store_vals, np.float32)
    Wq, Wk, Wv, Wo, Wkk, Wproj = (np.asarray(w, np.float32)
                                  for w in (Wq, Wk, Wv, Wo, Wkk, Wproj))
    bproj = np.asarray(bproj, np.float32)
    Wg = np.asarray(Wg, np.float32)
    bg = np.asarray(bg, np.float32)

    nc = _build()

    xtb = [np.ascontiguousarray(x[b].T) for b in range(B)]
    wqt = np.ascontiguousarray(Wq.T)
    wkt = np.ascontiguousarray(Wk.T)
    wvt = np.ascontiguousarray(Wv.T)
    wot = np.ascontiguousarray(Wo.T)
    wkkt = np.ascontiguousarray(Wkk.T)
    wpt = np.ascontiguousarray(Wproj.T)
    wgt = np.ascontiguousarray(Wg[0].reshape(2 * D, 1))
    bgt = bg.reshape(1, 1)
    bprojc = np.ascontiguousarray(bproj.reshape(D, 1))

    in_maps = []
    for c in range(NCORES):
        b, blk = c // 4, c % 4
        q0 = blk * 256
        validn = np.zeros((128, 3), np.float32)
        for ch in range(3):
            if (ch + 1) * 256 > q0:
                validn[:, ch] = NEG
        in_maps.append({
            "xt": xtb[b],
            "xtq": np.ascontiguousarray(xtb[b][:, q0:q0 + RPC]),
            "keys": store_keys[c * SS:(c + 1) * SS],
            "vals": store_vals[c * SS:(c + 1) * SS],
            "wqt": wqt, "wkt": wkt, "wvt": wvt, "wot": wot,
            "wkkt": wkkt, "wpt": wpt, "wgt": wgt, "bg": bgt,
            "bproj": bprojc, "validn": validn,
        })

    from concourse.bass_utils import run_bass_kernel_spmd
    trace = bool(os.environ.get("KNN_TRACE"))
    res = run_bass_kernel_spmd(nc, in_maps, list(range(NCORES)), trace=trace,
                               tmpdir=os.environ.get("KNN_TRACE_DIR"))
    if trace:
        _BUILT["exec_time_ns"] = res.exec_time_ns
    out = np.concatenate([res.results[c]["out"] for c in range(NCORES)], axis=0)
    return out.reshape(B, S, D).astype(np.float32)
